# revision 46
# baseline (speedup 1.0000x reference)
"""Trainium2 Bass kernel for nn_BaseNeuron (1-D stencil dz/dt + elementwise H).

Self-contained: hardcodes shapes/sharding; distributes the M grid dimension
across 8 NeuronCores (data parallel, halo built host-side).

Layout: the grid is split into blocks of B=124 consecutive points; each
block is one SBUF *column* spanning partitions 0..123, with 4 halo
partitions so a column holds 128 consecutive z values. Neighbor shifts
along the grid are *partition* shifts, which compute engines cannot do
(APs must start at partition 0) -- so every cross-partition term is a
banded 128x128 matmul on the (otherwise idle) tensor engine, and all
DVE/ACT ops read their operands at offset 0 (fp16 2x mode stays enabled).

Per column (p = partition, all from z via PE into PSUM fp32):
  apb[p] = z[p+2] - z[p]            ( = a+b of the limiter pair)
  amb[p] = z[p+2] - 2z[p+1] + z[p]  ( = a-b)
  q1 = |apb|, q2 = |amb|  (ACT drains PSUM -> SBUF fp16)
  4*min(|a|,|b|) = 2*(max(q1,q2)-min(q1,q2)) = 2*|q1-q2|
  u = min(q1, 2*|q1-q2|) = 2*limiter(a,b)      (DVE, all fp16 aligned)
  dz[p] = -coef*(u[p+1]-u[p]) - 2*(z[p+2]-z[p+1]) - S[p]
        = PSUM accumulation of 3 banded matmuls (u, z, S), ACT-drained.
  H = C1 + KH*relu(-c2*dVdt)   (dense elementwise, DVE tensor_scalar)

h_function: delta_V = max(VT-V,-1) == -1 for every V > -54 (randn never
gets there), making the T-dependent factors constants; elements with
V < -54 get an exact host-side fixup. dz[0], dz[1], dz[M-1] use different
formulas; fixed up exactly on host. Device I/O is fp16 (optionally fp8e3
for Sourse/dVdt): the kernel is memory-bound and the 2e-2 rel-err budget
dwarfs the quantization noise (measured ~3e-4 fp16 / ~5e-3 with fp8).
"""

import math

import numpy as np

# ---------------- problem constants (hardcoded) ----------------
M = 33554432
NCORES = 8
P = 128
L = M // NCORES  # 4194304 grid points per core
B = 124  # grid points owned per column (plus 4 halo partitions = 128)
NREAL = -(-L // B)  # 33826 columns needed per core (last mostly overhang)
NT_S = 8  # stencil DMA tiles per core
TWS = 4232  # stencil tile width; N = 8*4232 = 33856 (30 pad columns)
N = NT_S * TWS
NT_D = 8  # dense (H) DMA tiles per core
TWD = 4096  # dense tile width; CD = 8*4096 = 32768
WPE = 512  # PSUM tile width (1 fp32 bank = one matmul)
CD = L // P  # 32768 dense columns per core for the elementwise H sweep

DT = 0.1
DTS = 0.5
VT = -55.0
SIGMA = 3.0
TAU_M = 10.0
SQRT2 = 1.4142135623730951
SQRT_2_PI = 0.7978845608028654

_f16 = np.float16
_f32 = np.float32

_COEF32 = _f32(0.5 * (1.0 - DT / DTS))
_C2_64 = -1.0 / SIGMA / SQRT2
_C2 = _f32(_C2_64)

_T32 = _f32(_f32(_f32(-1.0) / _f32(3.0)) / _f32(SQRT2))
_T64 = float(_T32)
_A64 = math.exp(
    0.0061 - 1.12 * _T64 - 0.257 * _T64**2 - 0.072 * _T64**3 - 0.0117 * _T64**4
)
_FT64 = SQRT_2_PI * math.exp(-(_T64**2)) / (1.00000001 + math.erf(_T64))
_C1 = float(_f32(_A64 / TAU_M))  # H = C1 + KH * relu(-c2*dVdt)
_KH = float(_f32(SQRT2 * _FT64))

_CACHE: dict = {}

# Shipping configuration.
_SHIP = dict(
    bf16=False, s8=True, dv8=True, ho8=True, abs_stt=True, dz_eng="act",
    h_eng="dve", iobufs=3, midbufs=2, outbufs=2,
)


def _np_dt(name: str):
    import ml_dtypes

    return {
        "f16": np.float16,
        "bf16": ml_dtypes.bfloat16,
        "f8e3": ml_dtypes.float8_e3m4,
    }[name]


def _tiles(total: int, w: int):
    out = []
    lo = 0
    while lo < total:
        out.append((lo, min(w, total - lo)))
        lo += w
    return out


def _weights_np(s8: bool) -> np.ndarray:
    """[128, 5*128] fp16 stationary matrices: W_apb, W_amb, W_u, W_z, W_s.

    lhsT convention: out[po] = sum_pi W[pi, po] * rhs[pi].
    """
    coef = float(_COEF32)
    W = np.zeros((P, 5 * P), np.float32)
    wa = W[:, 0:128]  # apb[po] = z[po+2] - z[po],       po < 125
    wb = W[:, 128:256]  # amb[po] = z[po+2] - 2z[po+1] + z[po], po < 125
    wu = W[:, 256:384]  # acc += coef*u[po] - coef*u[po+1],   po < 124
    wz = W[:, 384:512]  # acc += 2z[po+1] - 2z[po+2],         po < 124
    ws = W[:, 512:640]  # acc += -S[po],                      po < 124
    for po in range(125):
        wa[po + 2, po] += 1.0
        wa[po, po] += -1.0
        wb[po + 2, po] += 1.0
        wb[po + 1, po] += -2.0
        wb[po, po] += 1.0
    for po in range(B):
        wu[po, po] += coef
        wu[po + 1, po] += -coef
        wz[po + 1, po] += 2.0
        wz[po + 2, po] += -2.0
        ws[po, po] += -1.0
    return W.astype(np.float16)


def _build(
    reps: int = 1,
    bf16: bool = False,
    s8: bool = False,
    dv8: bool = False,
    ho8: bool = False,
    abs_stt: bool = True,
    dz_eng: str = "act",
    h_eng: str = "dve",
    iobufs: int = 3,
    midbufs: int = 2,
    outbufs: int = 3,
    dma_only: bool = False,
    stage: int = 99,
):
    """Build + compile the per-core Bass module (transposed 124-block grid).

    reps > 1 wraps the whole sweep in a hardware For_i loop (bench only).
    s8/dv8/ho8: float8e3 (e3m4) for the Sourse input / dVdt input / H output
    abs_stt: |q1-q2| via STT (x*-1 max x), 3 DVE ops (else max/min, 4 ops)
    dz_eng: engine for the dz PSUM->SBUF drain: act | dve | gpsimd
    h_eng: engine pair for the H ops: dve | act | mixed

    DMA layout/queues: DRAM tensors are tile-blocked ([nt, P, w] with each
    tile contiguous). HWDGE rings (sync/scalar) skew ~55% of descriptors
    onto 4 of the 16 SDMA engines, so the bulk loads go through gpsimd
    (SWDGE, perfectly even engine spread, ~3us Q7 cost per dma_start):
    zt/dvt/ho on gpsimd, st on sync, dz on scalar.
    """
    import contextlib

    import concourse.bacc as bacc
    import concourse.mybir as mybir
    from concourse.tile import TileContext

    f16 = mybir.dt.float16
    fz = mybir.dt.bfloat16 if bf16 else f16
    f8 = mybir.dt.float8e3
    dt_s = f8 if s8 else f16
    dt_dv = f8 if dv8 else f16
    dt_ho = f8 if ho8 else f16
    Alu = mybir.AluOpType
    Act = mybir.ActivationFunctionType

    nc = bacc.Bacc(
        "TRN2",
        target_bir_lowering=False,
        debug=False,
        enable_asserts=False,
        name="base_neuron_t",
    )
    zt_d = nc.dram_tensor("zt", [NT_S, P, TWS], fz, kind="ExternalInput")
    st_d = nc.dram_tensor("st", [NT_S, B, TWS], dt_s, kind="ExternalInput")
    dv_d = nc.dram_tensor("dvt", [NT_D, P, TWD], dt_dv, kind="ExternalInput")
    wt_d = nc.dram_tensor("wts", [P, 5 * P], fz, kind="ExternalInput")
    dz_d = nc.dram_tensor("dz", [NT_S, B, TWS], f16, kind="ExternalOutput")
    ho_d = nc.dram_tensor("ho", [NT_D, P, TWD], dt_ho, kind="ExternalOutput")

    stiles = [(t, TWS) for t in range(NT_S)]
    dtiles = [(t, TWD) for t in range(NT_D)]
    nt = max(len(stiles), len(dtiles))

    KAPPA = float(-_C2)  # relu scale for H

    with TileContext(nc) as tc:
        with (
            tc.tile_pool(name="const", bufs=1) as constp,
            tc.tile_pool(name="io", bufs=iobufs) as iop,
            tc.tile_pool(name="mid", bufs=midbufs) as mid,
            tc.tile_pool(name="out", bufs=outbufs) as outp,
            tc.tile_pool(name="psab", bufs=2, space="PSUM") as psab,
            tc.tile_pool(name="psc", bufs=4, space="PSUM") as psc,
        ):
            wts = constp.tile([P, 5 * P], fz, tag="wts")
            nc.sync.dma_start(out=wts[:, :], in_=wt_d[:, :])
            w_apb = wts[:, 0:128]
            w_amb = wts[:, 128:256]
            w_u = wts[:, 256:384]
            w_z = wts[:, 384:512]
            w_s = wts[:, 512:640]

            live: dict = {}

            def phase12(t):
                """DMA in + apb/amb matmuls + ACT abs + DVE u for tile t."""
                _, w = stiles[t]
                zt = iop.tile([P, w], fz, tag="zt")
                nc.gpsimd.dma_start(out=zt[:, :], in_=zt_d[t, :, :])
                st = iop.tile([B, w], dt_s, tag="st")
                nc.sync.dma_start(out=st[:, :], in_=st_d[t, :, :])

                if dma_only:
                    dzt = outp.tile([B, w], f16, tag="dzt")
                    nc.vector.tensor_copy(dzt[:, :], st[0:B, :])
                    nc.gpsimd.dma_start(out=dz_d[t, :, :], in_=dzt[:, :])
                    return

                subs = _tiles(w, WPE)
                q1 = mid.tile([125, w], f16, tag="q1")
                q2 = mid.tile([125, w], f16, tag="q2")
                # interleave A/B so PE ping-pongs with ACT drains
                for c, sw in subs:
                    pa = psab.tile([125, sw], mybir.dt.float32, tag="pA")
                    nc.tensor.matmul(
                        pa[:, :], w_apb[0:127, 0:125], zt[0:127, c : c + sw],
                        start=True, stop=True,
                    )
                    nc.scalar.activation(q1[:, c : c + sw], pa[:, :], Act.Abs)
                    if stage < 2:
                        continue
                    pb = psab.tile([125, sw], mybir.dt.float32, tag="pB")
                    nc.tensor.matmul(
                        pb[:, :], w_amb[0:127, 0:125], zt[0:127, c : c + sw],
                        start=True, stop=True,
                    )
                    nc.scalar.activation(q2[:, c : c + sw], pb[:, :], Act.Abs)
                if stage <= 2:
                    dzt = outp.tile([B, w], f16, tag="dzt")
                    nc.vector.tensor_copy(
                        dzt[:, :], q1[0:B, :] if stage == 1 else q2[0:B, :]
                    )
                    nc.gpsimd.dma_start(out=dz_d[t, :, :], in_=dzt[:, :])
                    return
                # u = min(q1, 2|q1-q2|) on DVE (all fp16, offset-0 -> 2x;
                # STT runs at 1x on HW so only tensor_tensor/tensor_scalar)
                s_ = mid.tile([125, w], f16, tag="s")
                nc.vector.tensor_tensor(
                    s_[:, :], q1[:, :], q2[:, :], Alu.subtract
                )
                p2 = mid.tile([125, w], f16, tag="p2")
                nc.vector.tensor_scalar(
                    p2[:, :], s_[:, :], 2.0, None, Alu.mult
                )
                nc.vector.tensor_scalar(
                    s_[:, :], s_[:, :], -2.0, None, Alu.mult
                )
                # a2 = 2|q1-q2| = max(2s, -2s), in place over p2
                nc.vector.tensor_tensor(p2[:, :], p2[:, :], s_[:, :], Alu.max)
                u = mid.tile([125, w], fz, tag="u")
                nc.vector.tensor_tensor(u[:, :], q1[:, :], p2[:, :], Alu.min)
                live[t] = (w, zt, st, u)
                if stage == 3:
                    dzt = outp.tile([B, w], f16, tag="dzt")
                    nc.vector.tensor_copy(dzt[:, :], u[0:B, :])
                    nc.gpsimd.dma_start(out=dz_d[t, :, :], in_=dzt[:, :])
                    del live[t]

            def phase3(t):
                """dz accumulation matmuls + drain + DMA out for tile t."""
                if t not in live:
                    return
                w, zt, st, u = live.pop(t)
                subs = _tiles(w, WPE)
                dzt = outp.tile([B, w], f16, tag="dzt")
                fuse_s = dz_eng == "pool_sub"
                mats = [(w_u, 125, u), (w_z, 126, zt)]
                if not fuse_s:
                    mats.append((w_s, 124, st))
                pc_l = []
                for ci in range(0, len(subs), 4):
                    chunk = []
                    for c, sw in subs[ci : ci + 4]:
                        pc = psc.tile([B, sw], mybir.dt.float32, tag="pC")
                        chunk.append((c, sw, pc))
                    for mi, (wmat, wk, op) in enumerate(mats):
                        for c, sw, pc in chunk:
                            nc.tensor.matmul(
                                pc[:, :],
                                wmat[0:wk, 0:124],
                                op[0:wk, c : c + sw],
                                start=mi == 0, stop=mi == len(mats) - 1,
                            )
                    pc_l.extend(chunk)
                for c, sw, pc in pc_l:
                    if fuse_s:
                        # dz = pC - S on the (otherwise idle) gpsimd engine
                        nc.gpsimd.scalar_tensor_tensor(
                            dzt[:, c : c + sw], pc[:, :], 1.0,
                            st[0:B, c : c + sw], Alu.mult, Alu.subtract,
                        )
                    elif dz_eng == "act":
                        nc.scalar.activation(
                            dzt[:, c : c + sw], pc[:, :], Act.Copy
                        )
                    elif dz_eng == "dve":
                        nc.vector.tensor_copy(dzt[:, c : c + sw], pc[:, :])
                    else:
                        nc.gpsimd.tensor_copy(dzt[:, c : c + sw], pc[:, :])
                nc.gpsimd.dma_start(out=dz_d[t, :, :], in_=dzt[:, :])

            with (
                tc.For_i(0, reps, 1) if reps > 1 else contextlib.nullcontext()
            ):
                for t in range(nt + 1):
                    if t < len(stiles):
                        phase12(t)
                    if 0 <= t - 1 < len(stiles) and stage >= 4:
                        phase3(t - 1)

                    if t < len(dtiles):
                        _, w = dtiles[t]
                        dvt = iop.tile([P, w], dt_dv, tag="dvt")
                        nc.gpsimd.dma_start(out=dvt[:, :], in_=dv_d[t, :, :])
                        ht = outp.tile([P, w], dt_ho, tag="ht")
                        if dma_only or stage < 5:
                            nc.scalar.activation(ht[:, :], dvt[:, :], Act.Copy)
                        elif h_eng == "act":
                            rt = mid.tile([P, w], f16, tag="rt")
                            nc.scalar.activation(
                                rt[:, :], dvt[:, :], Act.Relu, scale=KAPPA
                            )
                            nc.scalar.activation(
                                ht[:, :], rt[:, :], Act.Copy, bias=_C1, scale=_KH
                            )
                        elif h_eng == "mixed":
                            rt = mid.tile([P, w], f16, tag="rt")
                            nc.vector.tensor_scalar(
                                rt[:, :], dvt[:, :], KAPPA, 0.0, Alu.mult, Alu.max
                            )
                            nc.scalar.activation(
                                ht[:, :], rt[:, :], Act.Copy, bias=_C1, scale=_KH
                            )
                        else:
                            rt = mid.tile([P, w], f16, tag="rt")
                            nc.vector.tensor_scalar(
                                rt[:, :], dvt[:, :], KAPPA, 0.0, Alu.mult, Alu.max
                            )
                            nc.vector.tensor_scalar(
                                ht[:, :], rt[:, :], _KH, _C1, Alu.mult, Alu.add
                            )
                        nc.gpsimd.dma_start(out=ho_d[t, :, :], in_=ht[:, :])

    nc.compile()
    return nc


def _make_sharded(nc, donate: bool = True):
    """Build the shard_map-jitted callable for a compiled Bass module."""
    import jax
    import concourse.mybir as mybir
    from concourse.bass2jax import (
        _bass_exec_p,
        install_neuronx_cc_hook,
        partition_id_tensor,
    )
    from jax.experimental.shard_map import shard_map
    from jax.sharding import Mesh, PartitionSpec

    install_neuronx_cc_hook()

    in_names: list[str] = []
    out_names: list[str] = []
    out_avals = []
    for alloc in nc.m.functions[0].allocations:
        if not isinstance(alloc, mybir.MemoryLocationSet):
            continue
        name = alloc.memorylocations[0].name
        if alloc.kind == "ExternalInput":
            in_names.append(name)
        elif alloc.kind == "ExternalOutput":
            out_names.append(name)
            out_avals.append(
                jax.core.ShapedArray(
                    tuple(alloc.tensor_shape), mybir.dt.np(alloc.dtype)
                )
            )

    partition_name = nc.partition_id_tensor.name if nc.partition_id_tensor else None
    if partition_name is not None and partition_name in in_names:
        in_names.remove(partition_name)
    n_params = len(in_names)
    n_outs = len(out_names)
    all_names = list(in_names) + list(out_names)
    if partition_name is not None:
        all_names.append(partition_name)

    def _body(*args):
        operands = list(args)
        if partition_name is not None:
            operands.append(partition_id_tensor())
        outs = _bass_exec_p.bind(
            *operands,
            out_avals=tuple(out_avals),
            in_names=tuple(all_names),
            out_names=tuple(out_names),
            lowering_input_output_aliases=(),
            sim_require_finite=True,
            sim_require_nnan=True,
            nc=nc,
        )
        return tuple(outs)

    devices = jax.devices()[:NCORES]
    assert len(devices) == NCORES
    mesh = Mesh(np.asarray(devices), ("core",))
    in_specs = (PartitionSpec("core"),) * (n_params + n_outs)
    out_specs = (PartitionSpec("core"),) * n_outs
    donate_argnums = tuple(range(n_params, n_params + n_outs)) if donate else ()
    sharded = jax.jit(
        shard_map(
            _body, mesh=mesh, in_specs=in_specs, out_specs=out_specs, check_rep=False
        ),
        donate_argnums=donate_argnums,
        keep_unused=True,
    )

    return {
        "nc": nc,
        "sharded": sharded,
        "in_names": in_names,
        "out_names": out_names,
        "out_avals": out_avals,
        "n_params": n_params,
        "n_outs": n_outs,
        "partition_name": partition_name,
        "mesh": mesh,
    }


def _get_runner():
    if "runner" not in _CACHE:
        _CACHE["runner"] = _make_sharded(_build(**_SHIP))
    return _CACHE["runner"]


def _prep_arrays(z: np.ndarray, S: np.ndarray, dV: np.ndarray) -> dict:
    """Host-side shard prep: dtype casts + tile-blocked transposed layouts.

    zt [8*NT_S, P, TWS]: core k, tile t, partition p, col j ->
        z[k*L + B*(t*TWS + j) + p - 2]
    st [8*NT_S, B, TWS]: same mapping without the -2 halo offset
    dvt [8*NT_D, P, TWD]: dense row-major per core (H is elementwise)
    wts [8P, 5*128]: stationary matrices, replicated per core
    """
    from numpy.lib.stride_tricks import as_strided

    dt_s = _np_dt("f8e3" if _SHIP.get("s8") else "f16")
    dt_dv = _np_dt("f8e3" if _SHIP.get("dv8") else "f16")
    dt_z = _np_dt("bf16" if _SHIP.get("bf16") else "f16")

    z16 = z.astype(dt_z)
    zp = np.zeros(7 * L + B * N + 256, dt_z)
    zp[2 : 2 + M] = z16  # zp[j] = z[j-2], zeros outside
    zblocks = []
    for k in range(NCORES):
        v = as_strided(zp[k * L :], shape=(N, P), strides=(B * 2, 2))
        # [N, P] -> [NT_S, TWS, P] -> [NT_S, P, TWS]
        zblocks.append(
            np.ascontiguousarray(v.reshape(NT_S, TWS, P).transpose(0, 2, 1))
        )
    zt = np.concatenate(zblocks, axis=0)

    spad = np.zeros(7 * L + B * N + 256, np.float32)
    spad[:M] = S
    s8 = spad.astype(dt_s)
    sblocks = []
    for k in range(NCORES):
        v = s8[k * L : k * L + B * N].reshape(N, B)
        sblocks.append(
            np.ascontiguousarray(v.reshape(NT_S, TWS, B).transpose(0, 2, 1))
        )
    st = np.concatenate(sblocks, axis=0)

    dvt = np.ascontiguousarray(
        dV.astype(dt_dv).reshape(NCORES, P, NT_D, TWD).transpose(0, 2, 1, 3)
    ).reshape(NCORES * NT_D, P, TWD)
    wts = np.tile(_weights_np(bool(_SHIP.get("s8"))).astype(dt_z), (NCORES, 1))
    return {"zt": zt, "st": st, "dvt": dvt, "wts": wts}


def _limiter_scalar(a: np.float32, b: np.float32) -> np.float32:
    x1 = _f32(_f32(abs(_f32(a + b))) * _f32(0.5))
    x2 = _f32(_f32(2.0) * min(_f32(abs(a)), _f32(abs(b))))
    return min(x1, x2)


def _h_exact(v: np.ndarray, dv: np.ndarray) -> np.ndarray:
    """Exact fp32 replica of the reference h_function (for rare V<-54 fixups)."""
    v = v.astype(np.float32)
    dv = dv.astype(np.float32)
    delta_v = np.maximum(_f32(VT) - v, _f32(-1.0))
    T = (delta_v / _f32(SIGMA) / _f32(SQRT2)).astype(np.float32)
    T64 = T.astype(np.float64)
    A = np.exp(
        0.0061 - 1.12 * T64 - 0.257 * T64**2 - 0.072 * T64**3 - 0.0117 * T64**4
    ).astype(np.float32)
    dT_dt = np.minimum(_f32(_C2) * dv, _f32(0.0)).astype(np.float32)
    erf = np.vectorize(math.erf)(T64)
    F_T = (SQRT_2_PI * np.exp(-(T64**2)) / (1.00000001 + erf)).astype(np.float32)
    B_ = (_f32(-SQRT2) * dT_dt * F_T * _f32(TAU_M)).astype(np.float32)
    return np.maximum((A + B_) / _f32(TAU_M), _f32(0.0)).astype(np.float32)


def kernel(z, Sourse, V, dVdt) -> np.ndarray:
    z = np.ascontiguousarray(np.asarray(z, dtype=np.float32))
    S = np.ascontiguousarray(np.asarray(Sourse, dtype=np.float32))
    V = np.asarray(V, dtype=np.float32)
    dV = np.ascontiguousarray(np.asarray(dVdt, dtype=np.float32))
    assert z.shape == (M,)

    r = _get_runner()
    arrs = _prep_arrays(z, S, dV)
    ins = [arrs[name] for name in r["in_names"]]
    zeros = [
        np.zeros((NCORES * av.shape[0], *av.shape[1:]), av.dtype)
        for av in r["out_avals"]
    ]
    out_arrs = r["sharded"](*ins, *zeros)
    by_name = dict(zip(r["out_names"], out_arrs))

    out = np.empty((2, M), np.float32)
    dz_t = np.asarray(by_name["dz"])  # [8*NT_S, B, TWS] fp16
    for k in range(NCORES):
        blk = dz_t[k * NT_S : (k + 1) * NT_S]  # [NT_S, B, TWS]
        cols = blk.transpose(0, 2, 1).reshape(N, B)  # [global col, p]
        out[0, k * L : (k + 1) * L] = cols.reshape(-1)[:L].astype(np.float32)
    ho_t = np.asarray(by_name["ho"])  # [8*NT_D, P, TWD]
    out[1] = (
        ho_t.reshape(NCORES, NT_D, P, TWD)
        .transpose(0, 2, 1, 3)
        .reshape(M)
        .astype(np.float32)
    )

    # ---- exact host fixups for the 3 boundary dz elements ----
    z0, z1, z2_ = _f32(z[0]), _f32(z[1]), _f32(z[2])
    s0, s1 = _f32(S[0]), _f32(S[1])
    out[0, 0] = _f32(_f32(_f32(-2.0) * z0) - s0)
    d0 = _f32(z1 - z0)
    d1 = _f32(z2_ - z1)
    w1 = _limiter_scalar(d1, d0)
    t = _f32(_COEF32 * _f32(w1 - _f32(0.0)))
    out[0, 1] = _f32(_f32(_f32(-2.0) * _f32(d0 + t)) - s1)
    zm1, zm2, zm3 = _f32(z[M - 1]), _f32(z[M - 2]), _f32(z[M - 3])
    wl = _limiter_scalar(_f32(zm1 - zm2), _f32(zm2 - zm3))
    out[0, M - 1] = _f32(
        _f32(_f32(2.0) * _f32(zm2 + _f32(_COEF32 * wl))) - _f32(S[M - 1])
    )

    # ---- H fixup for any V < -54 (delta_V != -1); never triggers for randn ----
    bad = np.flatnonzero(V < _f32(-54.0))
    if bad.size:
        out[1, bad] = _h_exact(V[bad], dV[bad])

    return out


# revision 48
# speedup vs baseline: 1.0028x; 1.0028x over previous
"""Trainium2 Bass kernel for nn_BaseNeuron (1-D stencil dz/dt + elementwise H).

Self-contained: hardcodes shapes/sharding; distributes the M grid dimension
across 8 NeuronCores (data parallel, halo built host-side).

Layout: the grid is split into blocks of B=124 consecutive points; each
block is one SBUF *column* spanning partitions 0..123, with 4 halo
partitions so a column holds 128 consecutive z values. Neighbor shifts
along the grid are *partition* shifts, which compute engines cannot do
(APs must start at partition 0) -- so every cross-partition term is a
banded 128x128 matmul on the (otherwise idle) tensor engine, and all
DVE/ACT ops read their operands at offset 0 (fp16 2x mode stays enabled).

Per column (p = partition, all from z via PE into PSUM fp32):
  apb[p] = z[p+2] - z[p]            ( = a+b of the limiter pair)
  amb[p] = z[p+2] - 2z[p+1] + z[p]  ( = a-b)
  q1 = |apb|, q2 = |amb|  (ACT drains PSUM -> SBUF fp16)
  4*min(|a|,|b|) = 2*(max(q1,q2)-min(q1,q2)) = 2*|q1-q2|
  u = min(q1, 2*|q1-q2|) = 2*limiter(a,b)      (DVE, all fp16 aligned)
  dz[p] = -coef*(u[p+1]-u[p]) - 2*(z[p+2]-z[p+1]) - S[p]
        = PSUM accumulation of 3 banded matmuls (u, z, S), ACT-drained.
  H = C1 + KH*relu(-c2*dVdt)   (dense elementwise, DVE tensor_scalar)

h_function: delta_V = max(VT-V,-1) == -1 for every V > -54 (randn never
gets there), making the T-dependent factors constants; elements with
V < -54 get an exact host-side fixup. dz[0], dz[1], dz[M-1] use different
formulas; fixed up exactly on host. Device I/O is fp16 (optionally fp8e3
for Sourse/dVdt): the kernel is memory-bound and the 2e-2 rel-err budget
dwarfs the quantization noise (measured ~3e-4 fp16 / ~5e-3 with fp8).
"""

import math

import numpy as np

# ---------------- problem constants (hardcoded) ----------------
M = 33554432
NCORES = 8
P = 128
L = M // NCORES  # 4194304 grid points per core
B = 124  # grid points owned per column (plus 4 halo partitions = 128)
NREAL = -(-L // B)  # 33826 columns needed per core (last mostly overhang)
NT_S = 8  # stencil DMA tiles per core
TWS = 4232  # stencil tile width; N = 8*4232 = 33856 (30 pad columns)
N = NT_S * TWS
NT_D = 8  # dense (H) DMA tiles per core
TWD = 4096  # dense tile width; CD = 8*4096 = 32768
WPE = 512  # PSUM tile width (1 fp32 bank = one matmul)
CD = L // P  # 32768 dense columns per core for the elementwise H sweep

DT = 0.1
DTS = 0.5
VT = -55.0
SIGMA = 3.0
TAU_M = 10.0
SQRT2 = 1.4142135623730951
SQRT_2_PI = 0.7978845608028654

_f16 = np.float16
_f32 = np.float32

_COEF32 = _f32(0.5 * (1.0 - DT / DTS))
_C2_64 = -1.0 / SIGMA / SQRT2
_C2 = _f32(_C2_64)

_T32 = _f32(_f32(_f32(-1.0) / _f32(3.0)) / _f32(SQRT2))
_T64 = float(_T32)
_A64 = math.exp(
    0.0061 - 1.12 * _T64 - 0.257 * _T64**2 - 0.072 * _T64**3 - 0.0117 * _T64**4
)
_FT64 = SQRT_2_PI * math.exp(-(_T64**2)) / (1.00000001 + math.erf(_T64))
_C1 = float(_f32(_A64 / TAU_M))  # H = C1 + KH * relu(-c2*dVdt)
_KH = float(_f32(SQRT2 * _FT64))

_CACHE: dict = {}

# Shipping configuration.
_SHIP = dict(
    bf16=False, s8=True, dv8=True, ho8=True, abs_stt=True, dz_eng="act",
    h_eng="dve", iobufs=3, midbufs=2, outbufs=2,
)


def _np_dt(name: str):
    import ml_dtypes

    return {
        "f16": np.float16,
        "bf16": ml_dtypes.bfloat16,
        "f8e3": ml_dtypes.float8_e3m4,
    }[name]


def _tiles(total: int, w: int):
    out = []
    lo = 0
    while lo < total:
        out.append((lo, min(w, total - lo)))
        lo += w
    return out


def _weights_np(s8: bool) -> np.ndarray:
    """[128, 5*128] fp16 stationary matrices: W_apb, W_amb, W_u, W_z, W_s.

    lhsT convention: out[po] = sum_pi W[pi, po] * rhs[pi].
    """
    coef = float(_COEF32)
    W = np.zeros((P, 5 * P), np.float32)
    wa = W[:, 0:128]  # apb[po] = z[po+2] - z[po],       po < 125
    wb = W[:, 128:256]  # amb[po] = z[po+2] - 2z[po+1] + z[po], po < 125
    wu = W[:, 256:384]  # acc += coef*u[po] - coef*u[po+1],   po < 124
    wz = W[:, 384:512]  # acc += 2z[po+1] - 2z[po+2],         po < 124
    ws = W[:, 512:640]  # acc += -S[po],                      po < 124
    for po in range(125):
        wa[po + 2, po] += 1.0
        wa[po, po] += -1.0
        wb[po + 2, po] += 1.0
        wb[po + 1, po] += -2.0
        wb[po, po] += 1.0
    for po in range(B):
        wu[po, po] += coef
        wu[po + 1, po] += -coef
        wz[po + 1, po] += 2.0
        wz[po + 2, po] += -2.0
        ws[po, po] += -1.0
    return W.astype(np.float16)


def _build(
    reps: int = 1,
    bf16: bool = False,
    s8: bool = False,
    dv8: bool = False,
    ho8: bool = False,
    abs_stt: bool = True,
    dz_eng: str = "act",
    h_eng: str = "dve",
    iobufs: int = 3,
    midbufs: int = 2,
    outbufs: int = 3,
    dma_only: bool = False,
    stage: int = 99,
):
    """Build + compile the per-core Bass module (transposed 124-block grid).

    reps > 1 wraps the whole sweep in a hardware For_i loop (bench only).
    s8/dv8/ho8: float8e3 (e3m4) for the Sourse input / dVdt input / H output
    abs_stt: |q1-q2| via STT (x*-1 max x), 3 DVE ops (else max/min, 4 ops)
    dz_eng: engine for the dz PSUM->SBUF drain: act | dve | gpsimd
    h_eng: engine pair for the H ops: dve | act | mixed

    DMA layout/queues: DRAM tensors are tile-blocked ([nt, P, w] with each
    tile contiguous). HWDGE rings (sync/scalar) skew ~55% of descriptors
    onto 4 of the 16 SDMA engines, so the bulk loads go through gpsimd
    (SWDGE, perfectly even engine spread, ~3us Q7 cost per dma_start):
    zt/dvt/ho on gpsimd, st on sync, dz on scalar.
    """
    import contextlib

    import concourse.bacc as bacc
    import concourse.mybir as mybir
    from concourse.tile import TileContext

    f16 = mybir.dt.float16
    fz = mybir.dt.bfloat16 if bf16 else f16
    f8 = mybir.dt.float8e3
    dt_s = f8 if s8 else f16
    dt_dv = f8 if dv8 else f16
    dt_ho = f8 if ho8 else f16
    Alu = mybir.AluOpType
    Act = mybir.ActivationFunctionType

    nc = bacc.Bacc(
        "TRN2",
        target_bir_lowering=False,
        debug=False,
        enable_asserts=False,
        name="base_neuron_t",
    )
    zt_d = nc.dram_tensor("zt", [NT_S, P, TWS], fz, kind="ExternalInput")
    st_d = nc.dram_tensor("st", [NT_S, B, TWS], dt_s, kind="ExternalInput")
    dv_d = nc.dram_tensor("dvt", [NT_D, P, TWD], dt_dv, kind="ExternalInput")
    wt_d = nc.dram_tensor("wts", [P, 5 * P], fz, kind="ExternalInput")
    dz_d = nc.dram_tensor("dz", [NT_S, B, TWS], f16, kind="ExternalOutput")
    ho_d = nc.dram_tensor("ho", [NT_D, P, TWD], dt_ho, kind="ExternalOutput")

    stiles = [(t, TWS) for t in range(NT_S)]
    dtiles = [(t, TWD) for t in range(NT_D)]
    nt = max(len(stiles), len(dtiles))

    KAPPA = float(-_C2)  # relu scale for H

    with TileContext(nc) as tc:
        with (
            tc.tile_pool(name="const", bufs=1) as constp,
            tc.tile_pool(name="io", bufs=iobufs) as iop,
            tc.tile_pool(name="mid", bufs=midbufs) as mid,
            tc.tile_pool(name="out", bufs=outbufs) as outp,
            tc.tile_pool(name="psab", bufs=2, space="PSUM") as psab,
            tc.tile_pool(name="psc", bufs=2, space="PSUM") as psc,
        ):
            wts = constp.tile([P, 5 * P], fz, tag="wts")
            nc.sync.dma_start(out=wts[:, :], in_=wt_d[:, :])
            w_apb = wts[:, 0:128]
            w_amb = wts[:, 128:256]
            w_u = wts[:, 256:384]
            w_z = wts[:, 384:512]
            w_s = wts[:, 512:640]

            live: dict = {}

            def phase12(t):
                """DMA in + apb/amb matmuls + ACT abs + DVE u for tile t."""
                _, w = stiles[t]
                zt = iop.tile([P, w], fz, tag="zt")
                nc.gpsimd.dma_start(out=zt[:, :], in_=zt_d[t, :, :])
                st = iop.tile([B, w], dt_s, tag="st")
                nc.sync.dma_start(out=st[:, :], in_=st_d[t, :, :])

                if dma_only:
                    dzt = outp.tile([B, w], f16, tag="dzt")
                    nc.vector.tensor_copy(dzt[:, :], st[0:B, :])
                    nc.gpsimd.dma_start(out=dz_d[t, :, :], in_=dzt[:, :])
                    return

                subs = _tiles(w, WPE)
                nch = len(subs)
                # q12[:, ci, 0:512] = |apb|, q12[:, ci, 512:1024] = |amb| of
                # chunk ci: one paired 2-bank psum tile -> ONE 1024-wide ACT
                # drain per chunk (halves the ACT per-op overhead count).
                q12 = mid.tile([125, nch, 2 * WPE], f16, tag="q12")
                for ci, (c, sw) in enumerate(subs):
                    pq = psab.tile([125, 2 * WPE], mybir.dt.float32, tag="pq")
                    nc.tensor.matmul(
                        pq[:, 0:sw], w_apb[0:127, 0:125],
                        zt[0:127, c : c + sw], start=True, stop=True,
                    )
                    nc.tensor.matmul(
                        pq[:, WPE : WPE + sw], w_amb[0:127, 0:125],
                        zt[0:127, c : c + sw], start=True, stop=True,
                    )
                    if sw == WPE:
                        nc.scalar.activation(q12[:, ci, :], pq[:, :], Act.Abs)
                    else:  # tail: drain only the written halves
                        nc.scalar.activation(
                            q12[:, ci, 0:sw], pq[:, 0:sw], Act.Abs
                        )
                        nc.scalar.activation(
                            q12[:, ci, WPE : WPE + sw],
                            pq[:, WPE : WPE + sw], Act.Abs,
                        )
                if stage <= 2:
                    dzt = outp.tile([B, w], f16, tag="dzt")
                    nc.vector.tensor_copy(
                        dzt[:, :], q1[0:B, :] if stage == 1 else q2[0:B, :]
                    )
                    nc.gpsimd.dma_start(out=dz_d[t, :, :], in_=dzt[:, :])
                    return
                # u = min(q1, 2|q1-q2|) on DVE (all fp16, offset-0 -> 2x;
                # STT runs at 1x on HW so only tensor_tensor/tensor_scalar).
                # q1/q2 live interleaved in q12; strided 3D views keep the
                # last dim packed so 2x mode still triggers.
                s_ = mid.tile([125, nch, WPE], f16, tag="s")
                p2 = mid.tile([125, nch, WPE], f16, tag="p2")
                u = mid.tile([125, nch, WPE], fz, tag="u")
                nm = nch - 1
                tl = subs[-1][1]  # tail chunk width
                views = [
                    (
                        q12[:, 0:nm, 0:WPE], q12[:, 0:nm, WPE : 2 * WPE],
                        s_[:, 0:nm, :], p2[:, 0:nm, :], u[:, 0:nm, :],
                    ),
                    (
                        q12[:, nm, 0:tl], q12[:, nm, WPE : WPE + tl],
                        s_[:, nm, 0:tl], p2[:, nm, 0:tl], u[:, nm, 0:tl],
                    ),
                ]
                for vq1, vq2, vs, vp, vu in views:
                    nc.vector.tensor_tensor(vs, vq1, vq2, Alu.subtract)
                    nc.vector.tensor_scalar(vp, vs, 2.0, None, Alu.mult)
                    nc.vector.tensor_scalar(vs, vs, -2.0, None, Alu.mult)
                    nc.vector.tensor_tensor(vp, vp, vs, Alu.max)
                    nc.vector.tensor_tensor(vu, vq1, vp, Alu.min)
                live[t] = (w, zt, st, u)
                if stage == 3:
                    dzt = outp.tile([B, w], f16, tag="dzt")
                    nc.vector.tensor_copy(dzt[:, 0:WPE], u[0:B, 0, :])
                    nc.gpsimd.dma_start(out=dz_d[t, :, :], in_=dzt[:, :])
                    del live[t]

            def phase3(t):
                """dz accumulation matmuls + drain + DMA out for tile t."""
                if t not in live:
                    return
                w, zt, st, u = live.pop(t)
                subs = _tiles(w, WPE)
                dzt = outp.tile([B, w], f16, tag="dzt")
                mats = [(w_u, 125, None), (w_z, 126, zt), (w_s, 124, st)]
                drains = []
                for ci0 in range(0, len(subs), 2):
                    pair = subs[ci0 : ci0 + 2]
                    pw = sum(sw for _, sw in pair)
                    pc = psc.tile([B, 2 * WPE], mybir.dt.float32, tag="pC")
                    for mi, (wmat, wk, op) in enumerate(mats):
                        for pi, (c, sw) in enumerate(pair):
                            rhs = (
                                u[0:wk, ci0 + pi, 0:sw] if op is None
                                else op[0:wk, c : c + sw]
                            )
                            nc.tensor.matmul(
                                pc[:, pi * WPE : pi * WPE + sw],
                                wmat[0:wk, 0:124], rhs,
                                start=mi == 0, stop=mi == len(mats) - 1,
                            )
                    drains.append((pair[0][0], pw, pc))
                for c, pw, pc in drains:
                    if dz_eng == "act":
                        nc.scalar.activation(
                            dzt[:, c : c + pw], pc[:, 0:pw], Act.Copy
                        )
                    elif dz_eng == "dve":
                        nc.vector.tensor_copy(dzt[:, c : c + pw], pc[:, 0:pw])
                    else:
                        nc.gpsimd.tensor_copy(dzt[:, c : c + pw], pc[:, 0:pw])
                nc.gpsimd.dma_start(out=dz_d[t, :, :], in_=dzt[:, :])

            with (
                tc.For_i(0, reps, 1) if reps > 1 else contextlib.nullcontext()
            ):
                for t in range(nt + 1):
                    if t < len(stiles):
                        phase12(t)
                    if 0 <= t - 1 < len(stiles) and stage >= 4:
                        phase3(t - 1)

                    if t < len(dtiles):
                        _, w = dtiles[t]
                        dvt = iop.tile([P, w], dt_dv, tag="dvt")
                        nc.gpsimd.dma_start(out=dvt[:, :], in_=dv_d[t, :, :])
                        ht = outp.tile([P, w], dt_ho, tag="ht")
                        if dma_only or stage < 5:
                            nc.scalar.activation(ht[:, :], dvt[:, :], Act.Copy)
                        elif h_eng == "act":
                            rt = mid.tile([P, w], f16, tag="rt")
                            nc.scalar.activation(
                                rt[:, :], dvt[:, :], Act.Relu, scale=KAPPA
                            )
                            nc.scalar.activation(
                                ht[:, :], rt[:, :], Act.Copy, bias=_C1, scale=_KH
                            )
                        elif h_eng == "mixed":
                            rt = mid.tile([P, w], f16, tag="rt")
                            nc.vector.tensor_scalar(
                                rt[:, :], dvt[:, :], KAPPA, 0.0, Alu.mult, Alu.max
                            )
                            nc.scalar.activation(
                                ht[:, :], rt[:, :], Act.Copy, bias=_C1, scale=_KH
                            )
                        else:
                            rt = mid.tile([P, w], f16, tag="rt")
                            nc.vector.tensor_scalar(
                                rt[:, :], dvt[:, :], KAPPA, 0.0, Alu.mult, Alu.max
                            )
                            nc.vector.tensor_scalar(
                                ht[:, :], rt[:, :], _KH, _C1, Alu.mult, Alu.add
                            )
                        nc.gpsimd.dma_start(out=ho_d[t, :, :], in_=ht[:, :])

    nc.compile()
    return nc


def _make_sharded(nc, donate: bool = True):
    """Build the shard_map-jitted callable for a compiled Bass module."""
    import jax
    import concourse.mybir as mybir
    from concourse.bass2jax import (
        _bass_exec_p,
        install_neuronx_cc_hook,
        partition_id_tensor,
    )
    from jax.experimental.shard_map import shard_map
    from jax.sharding import Mesh, PartitionSpec

    install_neuronx_cc_hook()

    in_names: list[str] = []
    out_names: list[str] = []
    out_avals = []
    for alloc in nc.m.functions[0].allocations:
        if not isinstance(alloc, mybir.MemoryLocationSet):
            continue
        name = alloc.memorylocations[0].name
        if alloc.kind == "ExternalInput":
            in_names.append(name)
        elif alloc.kind == "ExternalOutput":
            out_names.append(name)
            out_avals.append(
                jax.core.ShapedArray(
                    tuple(alloc.tensor_shape), mybir.dt.np(alloc.dtype)
                )
            )

    partition_name = nc.partition_id_tensor.name if nc.partition_id_tensor else None
    if partition_name is not None and partition_name in in_names:
        in_names.remove(partition_name)
    n_params = len(in_names)
    n_outs = len(out_names)
    all_names = list(in_names) + list(out_names)
    if partition_name is not None:
        all_names.append(partition_name)

    def _body(*args):
        operands = list(args)
        if partition_name is not None:
            operands.append(partition_id_tensor())
        outs = _bass_exec_p.bind(
            *operands,
            out_avals=tuple(out_avals),
            in_names=tuple(all_names),
            out_names=tuple(out_names),
            lowering_input_output_aliases=(),
            sim_require_finite=True,
            sim_require_nnan=True,
            nc=nc,
        )
        return tuple(outs)

    devices = jax.devices()[:NCORES]
    assert len(devices) == NCORES
    mesh = Mesh(np.asarray(devices), ("core",))
    in_specs = (PartitionSpec("core"),) * (n_params + n_outs)
    out_specs = (PartitionSpec("core"),) * n_outs
    donate_argnums = tuple(range(n_params, n_params + n_outs)) if donate else ()
    sharded = jax.jit(
        shard_map(
            _body, mesh=mesh, in_specs=in_specs, out_specs=out_specs, check_rep=False
        ),
        donate_argnums=donate_argnums,
        keep_unused=True,
    )

    return {
        "nc": nc,
        "sharded": sharded,
        "in_names": in_names,
        "out_names": out_names,
        "out_avals": out_avals,
        "n_params": n_params,
        "n_outs": n_outs,
        "partition_name": partition_name,
        "mesh": mesh,
    }


def _get_runner():
    if "runner" not in _CACHE:
        _CACHE["runner"] = _make_sharded(_build(**_SHIP))
    return _CACHE["runner"]


def _prep_arrays(z: np.ndarray, S: np.ndarray, dV: np.ndarray) -> dict:
    """Host-side shard prep: dtype casts + tile-blocked transposed layouts.

    zt [8*NT_S, P, TWS]: core k, tile t, partition p, col j ->
        z[k*L + B*(t*TWS + j) + p - 2]
    st [8*NT_S, B, TWS]: same mapping without the -2 halo offset
    dvt [8*NT_D, P, TWD]: dense row-major per core (H is elementwise)
    wts [8P, 5*128]: stationary matrices, replicated per core
    """
    from numpy.lib.stride_tricks import as_strided

    dt_s = _np_dt("f8e3" if _SHIP.get("s8") else "f16")
    dt_dv = _np_dt("f8e3" if _SHIP.get("dv8") else "f16")
    dt_z = _np_dt("bf16" if _SHIP.get("bf16") else "f16")

    z16 = z.astype(dt_z)
    zp = np.zeros(7 * L + B * N + 256, dt_z)
    zp[2 : 2 + M] = z16  # zp[j] = z[j-2], zeros outside
    zblocks = []
    for k in range(NCORES):
        v = as_strided(zp[k * L :], shape=(N, P), strides=(B * 2, 2))
        # [N, P] -> [NT_S, TWS, P] -> [NT_S, P, TWS]
        zblocks.append(
            np.ascontiguousarray(v.reshape(NT_S, TWS, P).transpose(0, 2, 1))
        )
    zt = np.concatenate(zblocks, axis=0)

    spad = np.zeros(7 * L + B * N + 256, np.float32)
    spad[:M] = S
    s8 = spad.astype(dt_s)
    sblocks = []
    for k in range(NCORES):
        v = s8[k * L : k * L + B * N].reshape(N, B)
        sblocks.append(
            np.ascontiguousarray(v.reshape(NT_S, TWS, B).transpose(0, 2, 1))
        )
    st = np.concatenate(sblocks, axis=0)

    dvt = np.ascontiguousarray(
        dV.astype(dt_dv).reshape(NCORES, P, NT_D, TWD).transpose(0, 2, 1, 3)
    ).reshape(NCORES * NT_D, P, TWD)
    wts = np.tile(_weights_np(bool(_SHIP.get("s8"))).astype(dt_z), (NCORES, 1))
    return {"zt": zt, "st": st, "dvt": dvt, "wts": wts}


def _limiter_scalar(a: np.float32, b: np.float32) -> np.float32:
    x1 = _f32(_f32(abs(_f32(a + b))) * _f32(0.5))
    x2 = _f32(_f32(2.0) * min(_f32(abs(a)), _f32(abs(b))))
    return min(x1, x2)


def _h_exact(v: np.ndarray, dv: np.ndarray) -> np.ndarray:
    """Exact fp32 replica of the reference h_function (for rare V<-54 fixups)."""
    v = v.astype(np.float32)
    dv = dv.astype(np.float32)
    delta_v = np.maximum(_f32(VT) - v, _f32(-1.0))
    T = (delta_v / _f32(SIGMA) / _f32(SQRT2)).astype(np.float32)
    T64 = T.astype(np.float64)
    A = np.exp(
        0.0061 - 1.12 * T64 - 0.257 * T64**2 - 0.072 * T64**3 - 0.0117 * T64**4
    ).astype(np.float32)
    dT_dt = np.minimum(_f32(_C2) * dv, _f32(0.0)).astype(np.float32)
    erf = np.vectorize(math.erf)(T64)
    F_T = (SQRT_2_PI * np.exp(-(T64**2)) / (1.00000001 + erf)).astype(np.float32)
    B_ = (_f32(-SQRT2) * dT_dt * F_T * _f32(TAU_M)).astype(np.float32)
    return np.maximum((A + B_) / _f32(TAU_M), _f32(0.0)).astype(np.float32)


def kernel(z, Sourse, V, dVdt) -> np.ndarray:
    z = np.ascontiguousarray(np.asarray(z, dtype=np.float32))
    S = np.ascontiguousarray(np.asarray(Sourse, dtype=np.float32))
    V = np.asarray(V, dtype=np.float32)
    dV = np.ascontiguousarray(np.asarray(dVdt, dtype=np.float32))
    assert z.shape == (M,)

    r = _get_runner()
    arrs = _prep_arrays(z, S, dV)
    ins = [arrs[name] for name in r["in_names"]]
    zeros = [
        np.zeros((NCORES * av.shape[0], *av.shape[1:]), av.dtype)
        for av in r["out_avals"]
    ]
    out_arrs = r["sharded"](*ins, *zeros)
    by_name = dict(zip(r["out_names"], out_arrs))

    out = np.empty((2, M), np.float32)
    dz_t = np.asarray(by_name["dz"])  # [8*NT_S, B, TWS] fp16
    for k in range(NCORES):
        blk = dz_t[k * NT_S : (k + 1) * NT_S]  # [NT_S, B, TWS]
        cols = blk.transpose(0, 2, 1).reshape(N, B)  # [global col, p]
        out[0, k * L : (k + 1) * L] = cols.reshape(-1)[:L].astype(np.float32)
    ho_t = np.asarray(by_name["ho"])  # [8*NT_D, P, TWD]
    out[1] = (
        ho_t.reshape(NCORES, NT_D, P, TWD)
        .transpose(0, 2, 1, 3)
        .reshape(M)
        .astype(np.float32)
    )

    # ---- exact host fixups for the 3 boundary dz elements ----
    z0, z1, z2_ = _f32(z[0]), _f32(z[1]), _f32(z[2])
    s0, s1 = _f32(S[0]), _f32(S[1])
    out[0, 0] = _f32(_f32(_f32(-2.0) * z0) - s0)
    d0 = _f32(z1 - z0)
    d1 = _f32(z2_ - z1)
    w1 = _limiter_scalar(d1, d0)
    t = _f32(_COEF32 * _f32(w1 - _f32(0.0)))
    out[0, 1] = _f32(_f32(_f32(-2.0) * _f32(d0 + t)) - s1)
    zm1, zm2, zm3 = _f32(z[M - 1]), _f32(z[M - 2]), _f32(z[M - 3])
    wl = _limiter_scalar(_f32(zm1 - zm2), _f32(zm2 - zm3))
    out[0, M - 1] = _f32(
        _f32(_f32(2.0) * _f32(zm2 + _f32(_COEF32 * wl))) - _f32(S[M - 1])
    )

    # ---- H fixup for any V < -54 (delta_V != -1); never triggers for randn ----
    bad = np.flatnonzero(V < _f32(-54.0))
    if bad.size:
        out[1, bad] = _h_exact(V[bad], dV[bad])

    return out


# revision 49
# speedup vs baseline: 1.0248x; 1.0219x over previous
"""Trainium2 Bass kernel for nn_BaseNeuron (1-D stencil dz/dt + elementwise H).

Self-contained: hardcodes shapes/sharding; distributes the M grid dimension
across 8 NeuronCores (data parallel, halo built host-side).

Layout: the grid is split into blocks of B=124 consecutive points; each
block is one SBUF *column* spanning partitions 0..123, with 4 halo
partitions so a column holds 128 consecutive z values. Neighbor shifts
along the grid are *partition* shifts, which compute engines cannot do
(APs must start at partition 0) -- so every cross-partition term is a
banded 128x128 matmul on the (otherwise idle) tensor engine, and all
DVE/ACT ops read their operands at offset 0 (fp16 2x mode stays enabled).

Per column (p = partition, all from z via PE into PSUM fp32):
  apb[p] = z[p+2] - z[p]            ( = a+b of the limiter pair)
  amb[p] = z[p+2] - 2z[p+1] + z[p]  ( = a-b)
  q1 = |apb|, q2 = |amb|  (ACT drains PSUM -> SBUF fp16)
  4*min(|a|,|b|) = 2*(max(q1,q2)-min(q1,q2)) = 2*|q1-q2|
  u = min(q1, 2*|q1-q2|) = 2*limiter(a,b)      (DVE, all fp16 aligned)
  dz[p] = -coef*(u[p+1]-u[p]) - 2*(z[p+2]-z[p+1]) - S[p]
        = PSUM accumulation of 3 banded matmuls (u, z, S), ACT-drained.
  H = C1 + KH*relu(-c2*dVdt)   (dense elementwise, DVE tensor_scalar)

h_function: delta_V = max(VT-V,-1) == -1 for every V > -54 (randn never
gets there), making the T-dependent factors constants; elements with
V < -54 get an exact host-side fixup. dz[0], dz[1], dz[M-1] use different
formulas; fixed up exactly on host. Device I/O is fp16 (optionally fp8e3
for Sourse/dVdt): the kernel is memory-bound and the 2e-2 rel-err budget
dwarfs the quantization noise (measured ~3e-4 fp16 / ~5e-3 with fp8).
"""

import math

import numpy as np

# ---------------- problem constants (hardcoded) ----------------
M = 33554432
NCORES = 8
P = 128
L = M // NCORES  # 4194304 grid points per core
B = 124  # grid points owned per column (plus 4 halo partitions = 128)
NREAL = -(-L // B)  # 33826 columns needed per core (last mostly overhang)
NT_S = 8  # stencil DMA tiles per core
TWS = 4232  # stencil tile width; N = 8*4232 = 33856 (30 pad columns)
N = NT_S * TWS
NT_D = 8  # dense (H) DMA tiles per core
TWD = 4096  # dense tile width; CD = 8*4096 = 32768
WPE = 512  # PSUM tile width (1 fp32 bank = one matmul)
CD = L // P  # 32768 dense columns per core for the elementwise H sweep

DT = 0.1
DTS = 0.5
VT = -55.0
SIGMA = 3.0
TAU_M = 10.0
SQRT2 = 1.4142135623730951
SQRT_2_PI = 0.7978845608028654

_f16 = np.float16
_f32 = np.float32

_COEF32 = _f32(0.5 * (1.0 - DT / DTS))
_C2_64 = -1.0 / SIGMA / SQRT2
_C2 = _f32(_C2_64)

_T32 = _f32(_f32(_f32(-1.0) / _f32(3.0)) / _f32(SQRT2))
_T64 = float(_T32)
_A64 = math.exp(
    0.0061 - 1.12 * _T64 - 0.257 * _T64**2 - 0.072 * _T64**3 - 0.0117 * _T64**4
)
_FT64 = SQRT_2_PI * math.exp(-(_T64**2)) / (1.00000001 + math.erf(_T64))
_C1 = float(_f32(_A64 / TAU_M))  # H = C1 + KH * relu(-c2*dVdt)
_KH = float(_f32(SQRT2 * _FT64))

_CACHE: dict = {}

# Shipping configuration.
_SHIP = dict(
    bf16=False, s8=True, dv8=True, ho8=True, abs_stt=True, dz_eng="act",
    h_eng="dve", iobufs=3, midbufs=2, outbufs=2,
)


def _np_dt(name: str):
    import ml_dtypes

    return {
        "f16": np.float16,
        "bf16": ml_dtypes.bfloat16,
        "f8e3": ml_dtypes.float8_e3m4,
    }[name]


def _tiles(total: int, w: int):
    out = []
    lo = 0
    while lo < total:
        out.append((lo, min(w, total - lo)))
        lo += w
    return out


def _weights_np(s8: bool) -> np.ndarray:
    """[128, 5*128] fp16 stationary matrices: W_apb, W_amb, W_u, W_z, W_s.

    lhsT convention: out[po] = sum_pi W[pi, po] * rhs[pi].
    """
    coef = float(_COEF32)
    W = np.zeros((P, 5 * P), np.float32)
    wa = W[:, 0:128]  # apb[po] = z[po+2] - z[po],       po < 125
    wb = W[:, 128:256]  # amb[po] = z[po+2] - 2z[po+1] + z[po], po < 125
    wu = W[:, 256:384]  # acc += coef*u[po] - coef*u[po+1],   po < 124
    wz = W[:, 384:512]  # acc += 2z[po+1] - 2z[po+2],         po < 124
    ws = W[:, 512:640]  # acc += -S[po],                      po < 124
    for po in range(125):
        wa[po + 2, po] += 1.0
        wa[po, po] += -1.0
        wb[po + 2, po] += 1.0
        wb[po + 1, po] += -2.0
        wb[po, po] += 1.0
    for po in range(B):
        wu[po, po] += coef
        wu[po + 1, po] += -coef
        wz[po + 1, po] += 2.0
        wz[po + 2, po] += -2.0
        ws[po, po] += -1.0
    return W.astype(np.float16)


def _build(
    reps: int = 1,
    bf16: bool = False,
    s8: bool = False,
    dv8: bool = False,
    ho8: bool = False,
    abs_stt: bool = True,
    dz_eng: str = "act",
    h_eng: str = "dve",
    iobufs: int = 3,
    midbufs: int = 2,
    outbufs: int = 3,
    dma_only: bool = False,
    stage: int = 99,
):
    """Build + compile the per-core Bass module (transposed 124-block grid).

    reps > 1 wraps the whole sweep in a hardware For_i loop (bench only).
    s8/dv8/ho8: float8e3 (e3m4) for the Sourse input / dVdt input / H output
    abs_stt: |q1-q2| via STT (x*-1 max x), 3 DVE ops (else max/min, 4 ops)
    dz_eng: engine for the dz PSUM->SBUF drain: act | dve | gpsimd
    h_eng: engine pair for the H ops: dve | act | mixed

    DMA layout/queues: DRAM tensors are tile-blocked ([nt, P, w] with each
    tile contiguous). HWDGE rings (sync/scalar) skew ~55% of descriptors
    onto 4 of the 16 SDMA engines, so the bulk loads go through gpsimd
    (SWDGE, perfectly even engine spread, ~3us Q7 cost per dma_start):
    zt/dvt/ho on gpsimd, st on sync, dz on scalar.
    """
    import contextlib

    import concourse.bacc as bacc
    import concourse.mybir as mybir
    from concourse.tile import TileContext

    f16 = mybir.dt.float16
    fz = mybir.dt.bfloat16 if bf16 else f16
    f8 = mybir.dt.float8e3
    dt_s = f8 if s8 else f16
    dt_dv = f8 if dv8 else f16
    dt_ho = f8 if ho8 else f16
    Alu = mybir.AluOpType
    Act = mybir.ActivationFunctionType

    nc = bacc.Bacc(
        "TRN2",
        target_bir_lowering=False,
        debug=False,
        enable_asserts=False,
        name="base_neuron_t",
    )
    zt_d = nc.dram_tensor("zt", [NT_S, P, TWS], fz, kind="ExternalInput")
    st_d = nc.dram_tensor("st", [NT_S, B, TWS], dt_s, kind="ExternalInput")
    dv_d = nc.dram_tensor("dvt", [NT_D, P, TWD], dt_dv, kind="ExternalInput")
    wt_d = nc.dram_tensor("wts", [P, 5 * P], fz, kind="ExternalInput")
    dz_d = nc.dram_tensor("dz", [NT_S, B, TWS], f16, kind="ExternalOutput")
    ho_d = nc.dram_tensor("ho", [NT_D, P, TWD], dt_ho, kind="ExternalOutput")

    stiles = [(t, TWS) for t in range(NT_S)]
    dtiles = [(t, TWD) for t in range(NT_D)]
    nt = max(len(stiles), len(dtiles))

    KAPPA = float(-_C2)  # relu scale for H

    with TileContext(nc) as tc:
        with (
            tc.tile_pool(name="const", bufs=1) as constp,
            tc.tile_pool(name="io", bufs=iobufs) as iop,
            tc.tile_pool(name="mid", bufs=midbufs) as mid,
            tc.tile_pool(name="out", bufs=outbufs) as outp,
            tc.tile_pool(name="psab", bufs=2, space="PSUM") as psab,
            tc.tile_pool(name="psc", bufs=2, space="PSUM") as psc,
        ):
            wts = constp.tile([P, 5 * P], fz, tag="wts")
            nc.sync.dma_start(out=wts[:, :], in_=wt_d[:, :])
            w_apb = wts[:, 0:128]
            w_amb = wts[:, 128:256]
            w_u = wts[:, 256:384]
            w_z = wts[:, 384:512]
            w_s = wts[:, 512:640]

            live: dict = {}

            def phase12(t):
                """DMA in + apb/amb matmuls + ACT abs + DVE u for tile t."""
                _, w = stiles[t]
                zt = iop.tile([P, w], fz, tag="zt")
                nc.gpsimd.dma_start(out=zt[:, :], in_=zt_d[t, :, :])
                st = iop.tile([B, w], dt_s, tag="st")
                nc.sync.dma_start(out=st[:, :], in_=st_d[t, :, :])

                if dma_only:
                    dzt = outp.tile([B, w], f16, tag="dzt")
                    nc.vector.tensor_copy(dzt[:, :], st[0:B, :])
                    nc.gpsimd.dma_start(out=dz_d[t, :, :], in_=dzt[:, :])
                    return

                subs = _tiles(w, WPE)
                nch = len(subs)
                # q12[:, ci, 0:512] = |apb|, q12[:, ci, 512:1024] = |amb| of
                # chunk ci: one paired 2-bank psum tile -> ONE 1024-wide ACT
                # drain per chunk (halves the ACT per-op overhead count).
                q12 = mid.tile([125, nch, 2 * WPE], f16, tag="q12")
                for ci, (c, sw) in enumerate(subs):
                    pq = psab.tile([125, 2 * WPE], mybir.dt.float32, tag="pq")
                    nc.tensor.matmul(
                        pq[:, 0:sw], w_apb[0:127, 0:125],
                        zt[0:127, c : c + sw], start=True, stop=True,
                    )
                    nc.tensor.matmul(
                        pq[:, WPE : WPE + sw], w_amb[0:127, 0:125],
                        zt[0:127, c : c + sw], start=True, stop=True,
                    )
                    if sw == WPE:
                        nc.scalar.activation(q12[:, ci, :], pq[:, :], Act.Abs)
                    else:  # tail: drain only the written halves
                        nc.scalar.activation(
                            q12[:, ci, 0:sw], pq[:, 0:sw], Act.Abs
                        )
                        nc.scalar.activation(
                            q12[:, ci, WPE : WPE + sw],
                            pq[:, WPE : WPE + sw], Act.Abs,
                        )
                if stage <= 2:
                    dzt = outp.tile([B, w], f16, tag="dzt")
                    nc.vector.tensor_copy(
                        dzt[:, :], q1[0:B, :] if stage == 1 else q2[0:B, :]
                    )
                    nc.gpsimd.dma_start(out=dz_d[t, :, :], in_=dzt[:, :])
                    return
                # u = min(q1, 2|q1-q2|) on DVE (all fp16, offset-0 -> 2x;
                # STT runs at 1x on HW so only tensor_tensor/tensor_scalar).
                # q1/q2 live interleaved in q12; strided 3D views keep the
                # last dim packed so 2x mode still triggers.
                s_ = mid.tile([125, nch, WPE], f16, tag="s")
                p2 = mid.tile([125, nch, WPE], f16, tag="p2")
                u = mid.tile([125, nch, WPE], fz, tag="u")
                nm = nch - 1
                tl = subs[-1][1]  # tail chunk width
                views = [
                    (
                        q12[:, 0:nm, 0:WPE], q12[:, 0:nm, WPE : 2 * WPE],
                        s_[:, 0:nm, :], p2[:, 0:nm, :], u[:, 0:nm, :],
                    ),
                    (
                        q12[:, nm, 0:tl], q12[:, nm, WPE : WPE + tl],
                        s_[:, nm, 0:tl], p2[:, nm, 0:tl], u[:, nm, 0:tl],
                    ),
                ]
                for vq1, vq2, vs, vp, vu in views:
                    nc.vector.tensor_tensor(vs, vq1, vq2, Alu.subtract)
                    nc.vector.tensor_scalar(vp, vs, 2.0, None, Alu.mult)
                    nc.vector.tensor_scalar(vs, vs, -2.0, None, Alu.mult)
                    nc.vector.tensor_tensor(vp, vp, vs, Alu.max)
                    nc.vector.tensor_tensor(vu, vq1, vp, Alu.min)
                live[t] = (w, zt, st, u)
                if stage == 3:
                    dzt = outp.tile([B, w], f16, tag="dzt")
                    nc.vector.tensor_copy(dzt[:, 0:WPE], u[0:B, 0, :])
                    nc.gpsimd.dma_start(out=dz_d[t, :, :], in_=dzt[:, :])
                    del live[t]

            def phase3(t):
                """dz accumulation matmuls + drain + DMA out for tile t."""
                if t not in live:
                    return
                w, zt, st, u = live.pop(t)
                subs = _tiles(w, WPE)
                dzt = outp.tile([B, w], f16, tag="dzt")
                mats = [(w_u, 125, None), (w_z, 126, zt), (w_s, 124, st)]
                drains = []
                for ci0 in range(0, len(subs), 2):
                    pair = subs[ci0 : ci0 + 2]
                    pw = sum(sw for _, sw in pair)
                    pc = psc.tile([B, 2 * WPE], mybir.dt.float32, tag="pC")
                    for mi, (wmat, wk, op) in enumerate(mats):
                        for pi, (c, sw) in enumerate(pair):
                            rhs = (
                                u[0:wk, ci0 + pi, 0:sw] if op is None
                                else op[0:wk, c : c + sw]
                            )
                            nc.tensor.matmul(
                                pc[:, pi * WPE : pi * WPE + sw],
                                wmat[0:wk, 0:124], rhs,
                                start=mi == 0, stop=mi == len(mats) - 1,
                            )
                    drains.append((pair[0][0], pw, pc))
                for c, pw, pc in drains:
                    if dz_eng == "act":
                        nc.scalar.activation(
                            dzt[:, c : c + pw], pc[:, 0:pw], Act.Copy
                        )
                    elif dz_eng == "dve":
                        nc.vector.tensor_copy(dzt[:, c : c + pw], pc[:, 0:pw])
                    else:
                        nc.gpsimd.tensor_copy(dzt[:, c : c + pw], pc[:, 0:pw])
                nc.gpsimd.dma_start(out=dz_d[t, :, :], in_=dzt[:, :])

            with (
                tc.For_i(0, reps, 1) if reps > 1 else contextlib.nullcontext()
            ):
                for t in range(nt + 1):
                    if t < len(stiles):
                        phase12(t)
                    if 0 <= t - 1 < len(stiles) and stage >= 4:
                        phase3(t - 1)

                    if t < len(dtiles):
                        _, w = dtiles[t]
                        dvt = iop.tile([P, w], dt_dv, tag="dvt")
                        nc.sync.dma_start(out=dvt[:, :], in_=dv_d[t, :, :])
                        ht = outp.tile([P, w], dt_ho, tag="ht")
                        if dma_only or stage < 5:
                            nc.scalar.activation(ht[:, :], dvt[:, :], Act.Copy)
                        elif h_eng == "act":
                            rt = mid.tile([P, w], f16, tag="rt")
                            nc.scalar.activation(
                                rt[:, :], dvt[:, :], Act.Relu, scale=KAPPA
                            )
                            nc.scalar.activation(
                                ht[:, :], rt[:, :], Act.Copy, bias=_C1, scale=_KH
                            )
                        elif h_eng == "mixed":
                            rt = mid.tile([P, w], f16, tag="rt")
                            nc.vector.tensor_scalar(
                                rt[:, :], dvt[:, :], KAPPA, 0.0, Alu.mult, Alu.max
                            )
                            nc.scalar.activation(
                                ht[:, :], rt[:, :], Act.Copy, bias=_C1, scale=_KH
                            )
                        else:
                            rt = mid.tile([P, w], f16, tag="rt")
                            nc.vector.tensor_scalar(
                                rt[:, :], dvt[:, :], KAPPA, 0.0, Alu.mult, Alu.max
                            )
                            nc.vector.tensor_scalar(
                                ht[:, :], rt[:, :], _KH, _C1, Alu.mult, Alu.add
                            )
                        nc.scalar.dma_start(out=ho_d[t, :, :], in_=ht[:, :])

    nc.compile()
    return nc


def _make_sharded(nc, donate: bool = True):
    """Build the shard_map-jitted callable for a compiled Bass module."""
    import jax
    import concourse.mybir as mybir
    from concourse.bass2jax import (
        _bass_exec_p,
        install_neuronx_cc_hook,
        partition_id_tensor,
    )
    from jax.experimental.shard_map import shard_map
    from jax.sharding import Mesh, PartitionSpec

    install_neuronx_cc_hook()

    in_names: list[str] = []
    out_names: list[str] = []
    out_avals = []
    for alloc in nc.m.functions[0].allocations:
        if not isinstance(alloc, mybir.MemoryLocationSet):
            continue
        name = alloc.memorylocations[0].name
        if alloc.kind == "ExternalInput":
            in_names.append(name)
        elif alloc.kind == "ExternalOutput":
            out_names.append(name)
            out_avals.append(
                jax.core.ShapedArray(
                    tuple(alloc.tensor_shape), mybir.dt.np(alloc.dtype)
                )
            )

    partition_name = nc.partition_id_tensor.name if nc.partition_id_tensor else None
    if partition_name is not None and partition_name in in_names:
        in_names.remove(partition_name)
    n_params = len(in_names)
    n_outs = len(out_names)
    all_names = list(in_names) + list(out_names)
    if partition_name is not None:
        all_names.append(partition_name)

    def _body(*args):
        operands = list(args)
        if partition_name is not None:
            operands.append(partition_id_tensor())
        outs = _bass_exec_p.bind(
            *operands,
            out_avals=tuple(out_avals),
            in_names=tuple(all_names),
            out_names=tuple(out_names),
            lowering_input_output_aliases=(),
            sim_require_finite=True,
            sim_require_nnan=True,
            nc=nc,
        )
        return tuple(outs)

    devices = jax.devices()[:NCORES]
    assert len(devices) == NCORES
    mesh = Mesh(np.asarray(devices), ("core",))
    in_specs = (PartitionSpec("core"),) * (n_params + n_outs)
    out_specs = (PartitionSpec("core"),) * n_outs
    donate_argnums = tuple(range(n_params, n_params + n_outs)) if donate else ()
    sharded = jax.jit(
        shard_map(
            _body, mesh=mesh, in_specs=in_specs, out_specs=out_specs, check_rep=False
        ),
        donate_argnums=donate_argnums,
        keep_unused=True,
    )

    return {
        "nc": nc,
        "sharded": sharded,
        "in_names": in_names,
        "out_names": out_names,
        "out_avals": out_avals,
        "n_params": n_params,
        "n_outs": n_outs,
        "partition_name": partition_name,
        "mesh": mesh,
    }


def _get_runner():
    if "runner" not in _CACHE:
        _CACHE["runner"] = _make_sharded(_build(**_SHIP))
    return _CACHE["runner"]


def _prep_arrays(z: np.ndarray, S: np.ndarray, dV: np.ndarray) -> dict:
    """Host-side shard prep: dtype casts + tile-blocked transposed layouts.

    zt [8*NT_S, P, TWS]: core k, tile t, partition p, col j ->
        z[k*L + B*(t*TWS + j) + p - 2]
    st [8*NT_S, B, TWS]: same mapping without the -2 halo offset
    dvt [8*NT_D, P, TWD]: dense row-major per core (H is elementwise)
    wts [8P, 5*128]: stationary matrices, replicated per core
    """
    from numpy.lib.stride_tricks import as_strided

    dt_s = _np_dt("f8e3" if _SHIP.get("s8") else "f16")
    dt_dv = _np_dt("f8e3" if _SHIP.get("dv8") else "f16")
    dt_z = _np_dt("bf16" if _SHIP.get("bf16") else "f16")

    z16 = z.astype(dt_z)
    zp = np.zeros(7 * L + B * N + 256, dt_z)
    zp[2 : 2 + M] = z16  # zp[j] = z[j-2], zeros outside
    zblocks = []
    for k in range(NCORES):
        v = as_strided(zp[k * L :], shape=(N, P), strides=(B * 2, 2))
        # [N, P] -> [NT_S, TWS, P] -> [NT_S, P, TWS]
        zblocks.append(
            np.ascontiguousarray(v.reshape(NT_S, TWS, P).transpose(0, 2, 1))
        )
    zt = np.concatenate(zblocks, axis=0)

    spad = np.zeros(7 * L + B * N + 256, np.float32)
    spad[:M] = S
    s8 = spad.astype(dt_s)
    sblocks = []
    for k in range(NCORES):
        v = s8[k * L : k * L + B * N].reshape(N, B)
        sblocks.append(
            np.ascontiguousarray(v.reshape(NT_S, TWS, B).transpose(0, 2, 1))
        )
    st = np.concatenate(sblocks, axis=0)

    dvt = np.ascontiguousarray(
        dV.astype(dt_dv).reshape(NCORES, P, NT_D, TWD).transpose(0, 2, 1, 3)
    ).reshape(NCORES * NT_D, P, TWD)
    wts = np.tile(_weights_np(bool(_SHIP.get("s8"))).astype(dt_z), (NCORES, 1))
    return {"zt": zt, "st": st, "dvt": dvt, "wts": wts}


def _limiter_scalar(a: np.float32, b: np.float32) -> np.float32:
    x1 = _f32(_f32(abs(_f32(a + b))) * _f32(0.5))
    x2 = _f32(_f32(2.0) * min(_f32(abs(a)), _f32(abs(b))))
    return min(x1, x2)


def _h_exact(v: np.ndarray, dv: np.ndarray) -> np.ndarray:
    """Exact fp32 replica of the reference h_function (for rare V<-54 fixups)."""
    v = v.astype(np.float32)
    dv = dv.astype(np.float32)
    delta_v = np.maximum(_f32(VT) - v, _f32(-1.0))
    T = (delta_v / _f32(SIGMA) / _f32(SQRT2)).astype(np.float32)
    T64 = T.astype(np.float64)
    A = np.exp(
        0.0061 - 1.12 * T64 - 0.257 * T64**2 - 0.072 * T64**3 - 0.0117 * T64**4
    ).astype(np.float32)
    dT_dt = np.minimum(_f32(_C2) * dv, _f32(0.0)).astype(np.float32)
    erf = np.vectorize(math.erf)(T64)
    F_T = (SQRT_2_PI * np.exp(-(T64**2)) / (1.00000001 + erf)).astype(np.float32)
    B_ = (_f32(-SQRT2) * dT_dt * F_T * _f32(TAU_M)).astype(np.float32)
    return np.maximum((A + B_) / _f32(TAU_M), _f32(0.0)).astype(np.float32)


def kernel(z, Sourse, V, dVdt) -> np.ndarray:
    z = np.ascontiguousarray(np.asarray(z, dtype=np.float32))
    S = np.ascontiguousarray(np.asarray(Sourse, dtype=np.float32))
    V = np.asarray(V, dtype=np.float32)
    dV = np.ascontiguousarray(np.asarray(dVdt, dtype=np.float32))
    assert z.shape == (M,)

    r = _get_runner()
    arrs = _prep_arrays(z, S, dV)
    ins = [arrs[name] for name in r["in_names"]]
    zeros = [
        np.zeros((NCORES * av.shape[0], *av.shape[1:]), av.dtype)
        for av in r["out_avals"]
    ]
    out_arrs = r["sharded"](*ins, *zeros)
    by_name = dict(zip(r["out_names"], out_arrs))

    out = np.empty((2, M), np.float32)
    dz_t = np.asarray(by_name["dz"])  # [8*NT_S, B, TWS] fp16
    for k in range(NCORES):
        blk = dz_t[k * NT_S : (k + 1) * NT_S]  # [NT_S, B, TWS]
        cols = blk.transpose(0, 2, 1).reshape(N, B)  # [global col, p]
        out[0, k * L : (k + 1) * L] = cols.reshape(-1)[:L].astype(np.float32)
    ho_t = np.asarray(by_name["ho"])  # [8*NT_D, P, TWD]
    out[1] = (
        ho_t.reshape(NCORES, NT_D, P, TWD)
        .transpose(0, 2, 1, 3)
        .reshape(M)
        .astype(np.float32)
    )

    # ---- exact host fixups for the 3 boundary dz elements ----
    z0, z1, z2_ = _f32(z[0]), _f32(z[1]), _f32(z[2])
    s0, s1 = _f32(S[0]), _f32(S[1])
    out[0, 0] = _f32(_f32(_f32(-2.0) * z0) - s0)
    d0 = _f32(z1 - z0)
    d1 = _f32(z2_ - z1)
    w1 = _limiter_scalar(d1, d0)
    t = _f32(_COEF32 * _f32(w1 - _f32(0.0)))
    out[0, 1] = _f32(_f32(_f32(-2.0) * _f32(d0 + t)) - s1)
    zm1, zm2, zm3 = _f32(z[M - 1]), _f32(z[M - 2]), _f32(z[M - 3])
    wl = _limiter_scalar(_f32(zm1 - zm2), _f32(zm2 - zm3))
    out[0, M - 1] = _f32(
        _f32(_f32(2.0) * _f32(zm2 + _f32(_COEF32 * wl))) - _f32(S[M - 1])
    )

    # ---- H fixup for any V < -54 (delta_V != -1); never triggers for randn ----
    bad = np.flatnonzero(V < _f32(-54.0))
    if bad.size:
        out[1, bad] = _h_exact(V[bad], dV[bad])

    return out


# revision 53
# speedup vs baseline: 1.3455x; 1.3130x over previous
"""Trainium2 Bass kernel for nn_BaseNeuron (1-D stencil dz/dt + elementwise H).

Self-contained: hardcodes shapes/sharding; distributes the M grid dimension
across 8 NeuronCores (data parallel, halo built host-side).

Layout: the grid is split into blocks of B=124 consecutive points; each
block is one SBUF *column* spanning partitions 0..123, with 4 halo
partitions so a column holds 128 consecutive z values. Neighbor shifts
along the grid are *partition* shifts, which compute engines cannot do
(APs must start at partition 0) -- so every cross-partition term is a
banded 128x128 matmul on the (otherwise idle) tensor engine, and all
DVE/ACT ops read their operands at offset 0 (fp16 2x mode stays enabled).

Per column (p = partition, all from z via PE into PSUM fp32):
  apb[p] = z[p+2] - z[p]            ( = a+b of the limiter pair)
  amb[p] = z[p+2] - 2z[p+1] + z[p]  ( = a-b)
  q1 = |apb|, q2 = |amb|  (ACT drains PSUM -> SBUF fp16)
  4*min(|a|,|b|) = 2*(max(q1,q2)-min(q1,q2)) = 2*|q1-q2|
  u = min(q1, 2*|q1-q2|) = 2*limiter(a,b)      (DVE, all fp16 aligned)
  dz[p] = -coef*(u[p+1]-u[p]) - 2*(z[p+2]-z[p+1]) - S[p]
        = PSUM accumulation of 3 banded matmuls (u, z, S), ACT-drained.
  H = C1 + KH*relu(-c2*dVdt)   (dense elementwise, DVE tensor_scalar)

h_function: delta_V = max(VT-V,-1) == -1 for every V > -54 (randn never
gets there), making the T-dependent factors constants; elements with
V < -54 get an exact host-side fixup. dz[0], dz[1], dz[M-1] use different
formulas; fixed up exactly on host. Device I/O is fp16 (optionally fp8e3
for Sourse/dVdt): the kernel is memory-bound and the 2e-2 rel-err budget
dwarfs the quantization noise (measured ~3e-4 fp16 / ~5e-3 with fp8).
"""

import math

import numpy as np

# ---------------- problem constants (hardcoded) ----------------
M = 33554432
NCORES = 8
P = 128
L = M // NCORES  # 4194304 grid points per core
B = 124  # grid points owned per column (plus 4 halo partitions = 128)
NREAL = -(-L // B)  # 33826 columns needed per core (last mostly overhang)
NT_S = 8  # stencil DMA tiles per core
TWS = 4232  # stencil tile width; N = 8*4232 = 33856 (30 pad columns)
N = NT_S * TWS
NT_D = 8  # dense (H) DMA tiles per core
TWD = 4096  # dense tile width; CD = 8*4096 = 32768
WPE = 512  # PSUM tile width (1 fp32 bank = one matmul)
CD = L // P  # 32768 dense columns per core for the elementwise H sweep

DT = 0.1
DTS = 0.5
VT = -55.0
SIGMA = 3.0
TAU_M = 10.0
SQRT2 = 1.4142135623730951
SQRT_2_PI = 0.7978845608028654

_f16 = np.float16
_f32 = np.float32

_COEF32 = _f32(0.5 * (1.0 - DT / DTS))
_C2_64 = -1.0 / SIGMA / SQRT2
_C2 = _f32(_C2_64)

_T32 = _f32(_f32(_f32(-1.0) / _f32(3.0)) / _f32(SQRT2))
_T64 = float(_T32)
_A64 = math.exp(
    0.0061 - 1.12 * _T64 - 0.257 * _T64**2 - 0.072 * _T64**3 - 0.0117 * _T64**4
)
_FT64 = SQRT_2_PI * math.exp(-(_T64**2)) / (1.00000001 + math.erf(_T64))
_C1 = float(_f32(_A64 / TAU_M))  # H = C1 + KH * relu(-c2*dVdt)
_KH = float(_f32(SQRT2 * _FT64))

_CACHE: dict = {}

# Shipping configuration.
_SHIP = dict(
    bf16=False, s8=True, dv8=True, ho8=True, abs_stt=True, dz_eng="act",
    h_eng="dve", iobufs=4, midbufs=2, outbufs=3,
)


def _np_dt(name: str):
    import ml_dtypes

    return {
        "f16": np.float16,
        "bf16": ml_dtypes.bfloat16,
        "f8e3": ml_dtypes.float8_e3m4,
    }[name]


def _tiles(total: int, w: int):
    out = []
    lo = 0
    while lo < total:
        out.append((lo, min(w, total - lo)))
        lo += w
    return out


def _weights_np(s8: bool) -> np.ndarray:
    """[128, 5*128] fp16 stationary matrices: W_apb, W_amb, W_u, W_z, W_s.

    lhsT convention: out[po] = sum_pi W[pi, po] * rhs[pi].
    """
    coef = float(_COEF32)
    W = np.zeros((P, 5 * P), np.float32)
    wa = W[:, 0:128]  # apb[po] = z[po+2] - z[po],       po < 125
    wb = W[:, 128:256]  # amb[po] = z[po+2] - 2z[po+1] + z[po], po < 125
    wu = W[:, 256:384]  # acc += coef*u[po] - coef*u[po+1],   po < 124
    wz = W[:, 384:512]  # acc += 2z[po+1] - 2z[po+2],         po < 124
    ws = W[:, 512:640]  # acc += -S[po],                      po < 124
    for po in range(125):
        wa[po + 2, po] += 1.0
        wa[po, po] += -1.0
        wb[po + 2, po] += 1.0
        wb[po + 1, po] += -2.0
        wb[po, po] += 1.0
    for po in range(B):
        wu[po, po] += coef
        wu[po + 1, po] += -coef
        wz[po + 1, po] += 2.0
        wz[po + 2, po] += -2.0
        ws[po, po] += -1.0
    return W.astype(np.float16)


def _build(
    reps: int = 1,
    bf16: bool = False,
    s8: bool = False,
    dv8: bool = False,
    ho8: bool = False,
    abs_stt: bool = True,
    dz_eng: str = "act",
    h_eng: str = "dve",
    iobufs: int = 3,
    midbufs: int = 2,
    outbufs: int = 2,
    dma_only: bool = False,
    stage: int = 99,
):
    """Build + compile the per-core Bass module (transposed 124-block grid).

    reps > 1 wraps the whole sweep in a hardware For_i loop (bench only).
    s8/dv8/ho8: float8e3 (e3m4) for the Sourse input / dVdt input / H output
    abs_stt: unused (STT measured 1x on HW; chain uses TT/tensor_scalar)
    dz_eng: engine for the dz PSUM->SBUF drain: act | dve | gpsimd
    h_eng: engine pair for the H ops: dve | act | mixed

    DMA layout/queues: DRAM tensors are tile-blocked ([nt, P, w] with each
    tile contiguous). HWDGE rings (sync/scalar) skew ~55% of descriptors
    onto 4 of the 16 SDMA engines, so the bulk loads go through gpsimd
    (SWDGE, perfectly even engine spread, ~1.4us Q7 cost per dma_start):
    zt/dz on gpsimd; st/dvt on sync and ho on scalar (HWDGE rings absorb
    ~8.5MB so the single SWDGE queue isn't the serial wall).
    """
    import contextlib

    import concourse.bacc as bacc
    import concourse.mybir as mybir
    from concourse.tile import TileContext

    f16 = mybir.dt.float16
    fz = mybir.dt.bfloat16 if bf16 else f16
    f8 = mybir.dt.float8e3
    dt_s = f8 if s8 else f16
    dt_dv = f8 if dv8 else f16
    dt_ho = f8 if ho8 else f16
    Alu = mybir.AluOpType
    Act = mybir.ActivationFunctionType

    nc = bacc.Bacc(
        "TRN2",
        target_bir_lowering=False,
        debug=False,
        enable_asserts=False,
        name="base_neuron_t",
    )
    zt_d = nc.dram_tensor("zt", [NT_S, P, TWS], fz, kind="ExternalInput")
    st_d = nc.dram_tensor("st", [NT_S, B, TWS], dt_s, kind="ExternalInput")
    dv_d = nc.dram_tensor("dvt", [NT_D, P, TWD], dt_dv, kind="ExternalInput")
    wt_d = nc.dram_tensor("wts", [P, 5 * P], fz, kind="ExternalInput")
    dz_d = nc.dram_tensor("dz", [NT_S, B, TWS], f16, kind="ExternalOutput")
    ho_d = nc.dram_tensor("ho", [NT_D, P, TWD], dt_ho, kind="ExternalOutput")

    stiles = [(t, TWS) for t in range(NT_S)]
    dtiles = [(t, TWD) for t in range(NT_D)]
    nt = max(len(stiles), len(dtiles))

    KAPPA = float(-_C2)  # relu scale for H

    with TileContext(nc) as tc:
        with (
            tc.tile_pool(name="const", bufs=1) as constp,
            tc.tile_pool(name="io", bufs=iobufs) as iop,
            tc.tile_pool(name="mid", bufs=midbufs) as mid,
            tc.tile_pool(name="scr", bufs=1) as scr,
            tc.tile_pool(name="out", bufs=outbufs) as outp,
            tc.tile_pool(name="psab", bufs=2, space="PSUM") as psab,
            tc.tile_pool(name="psc", bufs=2, space="PSUM") as psc,
        ):
            wts = constp.tile([P, 5 * P], fz, tag="wts")
            nc.sync.dma_start(out=wts[:, :], in_=wt_d[:, :])
            w_apb = wts[:, 0:128]
            w_amb = wts[:, 128:256]
            w_u = wts[:, 256:384]
            w_z = wts[:, 384:512]
            w_s = wts[:, 512:640]

            live: dict = {}

            def phase12(t):
                """DMA in + apb/amb matmuls + ACT abs + DVE u for tile t."""
                _, w = stiles[t]
                zt = iop.tile([P, w], fz, tag="zt")
                nc.gpsimd.dma_start(out=zt[:, :], in_=zt_d[t, :, :])
                st = iop.tile([B, w], dt_s, tag="st")
                nc.sync.dma_start(out=st[:, :], in_=st_d[t, :, :])

                if dma_only:
                    dzt = outp.tile([B, w], f16, tag="dzt")
                    nc.vector.tensor_copy(dzt[:, :], st[0:B, :])
                    nc.gpsimd.dma_start(out=dz_d[t, :, :], in_=dzt[:, :])
                    return

                subs = _tiles(w, WPE)
                nch = len(subs)
                # q12[:, ci, 0:512] = |apb|, q12[:, ci, 512:1024] = |amb| of
                # chunk ci: one paired 2-bank psum tile -> ONE 1024-wide ACT
                # drain per chunk (halves the ACT per-op overhead count).
                q12 = mid.tile([125, nch, 2 * WPE], f16, tag="q12")
                grp_pq = []
                for ci, (c, sw) in enumerate(subs):
                    if not grp_pq:
                        # group 2 chunks: A,A then B,B shares LDWEIGHTS
                        grp = [
                            (cj, cc, ss)
                            for cj, (cc, ss) in list(enumerate(subs))[ci : ci + 2]
                        ]
                        grp_pq = []
                        for cj, cc, ss in grp:
                            pq_t = psab.tile(
                                [125, 2 * WPE], mybir.dt.float32, tag="pq"
                            )
                            grp_pq.append((cj, cc, ss, pq_t))
                        for cj, cc, ss, pq_ in grp_pq:
                            nc.tensor.matmul(
                                pq_[:, 0:ss], w_apb[0:127, 0:125],
                                zt[0:127, cc : cc + ss],
                                start=True, stop=True,
                            )
                        for cj, cc, ss, pq_ in grp_pq:
                            nc.tensor.matmul(
                                pq_[:, WPE : WPE + ss], w_amb[0:127, 0:125],
                                zt[0:127, cc : cc + ss],
                                start=True, stop=True,
                            )
                    pq = grp_pq[0][3]
                    grp_pq = grp_pq[1:]
                    if sw == WPE:
                        nc.scalar.activation(q12[:, ci, :], pq[:, :], Act.Abs)
                    else:  # tail: drain only the written halves
                        nc.scalar.activation(
                            q12[:, ci, 0:sw], pq[:, 0:sw], Act.Abs
                        )
                        nc.scalar.activation(
                            q12[:, ci, WPE : WPE + sw],
                            pq[:, WPE : WPE + sw], Act.Abs,
                        )
                if stage <= 2:
                    dzt = outp.tile([B, w], f16, tag="dzt")
                    nc.vector.tensor_copy(
                        dzt[:, :], q1[0:B, :] if stage == 1 else q2[0:B, :]
                    )
                    nc.gpsimd.dma_start(out=dz_d[t, :, :], in_=dzt[:, :])
                    return
                # u = min(q1, 2|q1-q2|) on DVE (all fp16, offset-0 -> 2x;
                # STT runs at 1x on HW so only tensor_tensor/tensor_scalar).
                # q1/q2 live interleaved in q12; strided 3D views keep the
                # last dim packed so 2x mode still triggers.
                s_ = scr.tile([125, nch, WPE], f16, tag="s")
                p2 = scr.tile([125, nch, WPE], f16, tag="p2")
                u = mid.tile([125, nch, WPE], fz, tag="u")
                nm = nch - 1
                tl = subs[-1][1]  # tail chunk width
                views = [
                    (
                        q12[:, 0:nm, 0:WPE], q12[:, 0:nm, WPE : 2 * WPE],
                        s_[:, 0:nm, :], p2[:, 0:nm, :], u[:, 0:nm, :],
                    ),
                    (
                        q12[:, nm, 0:tl], q12[:, nm, WPE : WPE + tl],
                        s_[:, nm, 0:tl], p2[:, nm, 0:tl], u[:, nm, 0:tl],
                    ),
                ]
                for vq1, vq2, vs, vp, vu in views:
                    nc.vector.tensor_tensor(vs, vq1, vq2, Alu.subtract)
                    nc.vector.tensor_scalar(vp, vs, 2.0, None, Alu.mult)
                    nc.vector.tensor_scalar(vs, vs, -2.0, None, Alu.mult)
                    nc.vector.tensor_tensor(vp, vp, vs, Alu.max)
                    nc.vector.tensor_tensor(vu, vq1, vp, Alu.min)
                live[t] = (w, zt, st, u)
                if stage == 3:
                    dzt = outp.tile([B, w], f16, tag="dzt")
                    nc.vector.tensor_copy(dzt[:, 0:WPE], u[0:B, 0, :])
                    nc.gpsimd.dma_start(out=dz_d[t, :, :], in_=dzt[:, :])
                    del live[t]

            def phase3(t):
                """dz accumulation matmuls + drain + DMA out for tile t."""
                if t not in live:
                    return
                w, zt, st, u = live.pop(t)
                subs = _tiles(w, WPE)
                dzt = outp.tile([B, w], f16, tag="dzt")
                mats = [(w_u, 125, None), (w_z, 126, zt), (w_s, 124, st)]
                drains = []
                for ci0 in range(0, len(subs), 2):
                    pair = subs[ci0 : ci0 + 2]
                    pw = sum(sw for _, sw in pair)
                    pc = psc.tile([B, 2 * WPE], mybir.dt.float32, tag="pC")
                    for mi, (wmat, wk, op) in enumerate(mats):
                        for pi, (c, sw) in enumerate(pair):
                            rhs = (
                                u[0:wk, ci0 + pi, 0:sw] if op is None
                                else op[0:wk, c : c + sw]
                            )
                            nc.tensor.matmul(
                                pc[:, pi * WPE : pi * WPE + sw],
                                wmat[0:wk, 0:124], rhs,
                                start=mi == 0, stop=mi == len(mats) - 1,
                            )
                    drains.append((pair[0][0], pw, pc))
                for c, pw, pc in drains:
                    if dz_eng == "act":
                        nc.scalar.activation(
                            dzt[:, c : c + pw], pc[:, 0:pw], Act.Copy
                        )
                    elif dz_eng == "dve":
                        nc.vector.tensor_copy(dzt[:, c : c + pw], pc[:, 0:pw])
                    else:
                        nc.gpsimd.tensor_copy(dzt[:, c : c + pw], pc[:, 0:pw])
                nc.gpsimd.dma_start(out=dz_d[t, :, :], in_=dzt[:, :])

            with (
                tc.For_i(0, reps, 1) if reps > 1 else contextlib.nullcontext()
            ):
                for t in range(nt + 1):
                    if t < len(stiles):
                        phase12(t)
                    if 0 <= t - 1 < len(stiles) and stage >= 4:
                        phase3(t - 1)

                    if t < len(dtiles):
                        _, w = dtiles[t]
                        dvt = iop.tile([P, w], dt_dv, tag="dvt")
                        nc.sync.dma_start(out=dvt[:, :], in_=dv_d[t, :, :])
                        ht = outp.tile([P, w], dt_ho, tag="ht")
                        if dma_only or stage < 5:
                            nc.scalar.activation(ht[:, :], dvt[:, :], Act.Copy)
                        elif h_eng == "act":
                            rt = mid.tile([P, w], f16, tag="rt")
                            nc.scalar.activation(
                                rt[:, :], dvt[:, :], Act.Relu, scale=KAPPA
                            )
                            nc.scalar.activation(
                                ht[:, :], rt[:, :], Act.Copy, bias=_C1, scale=_KH
                            )
                        elif h_eng == "mixed":
                            rt = mid.tile([P, w], f16, tag="rt")
                            nc.vector.tensor_scalar(
                                rt[:, :], dvt[:, :], KAPPA, 0.0, Alu.mult, Alu.max
                            )
                            nc.scalar.activation(
                                ht[:, :], rt[:, :], Act.Copy, bias=_C1, scale=_KH
                            )
                        else:
                            rt = mid.tile([P, w], f16, tag="rt")
                            nc.vector.tensor_scalar(
                                rt[:, :], dvt[:, :], KAPPA, 0.0, Alu.mult, Alu.max
                            )
                            nc.vector.tensor_scalar(
                                ht[:, :], rt[:, :], _KH, _C1, Alu.mult, Alu.add
                            )
                        nc.scalar.dma_start(out=ho_d[t, :, :], in_=ht[:, :])

    nc.compile()
    return nc


def _make_sharded(nc, donate: bool = True):
    """Build the shard_map-jitted callable for a compiled Bass module."""
    import jax
    import concourse.mybir as mybir
    from concourse.bass2jax import (
        _bass_exec_p,
        install_neuronx_cc_hook,
        partition_id_tensor,
    )
    from jax.experimental.shard_map import shard_map
    from jax.sharding import Mesh, PartitionSpec

    install_neuronx_cc_hook()

    in_names: list[str] = []
    out_names: list[str] = []
    out_avals = []
    for alloc in nc.m.functions[0].allocations:
        if not isinstance(alloc, mybir.MemoryLocationSet):
            continue
        name = alloc.memorylocations[0].name
        if alloc.kind == "ExternalInput":
            in_names.append(name)
        elif alloc.kind == "ExternalOutput":
            out_names.append(name)
            out_avals.append(
                jax.core.ShapedArray(
                    tuple(alloc.tensor_shape), mybir.dt.np(alloc.dtype)
                )
            )

    partition_name = nc.partition_id_tensor.name if nc.partition_id_tensor else None
    if partition_name is not None and partition_name in in_names:
        in_names.remove(partition_name)
    n_params = len(in_names)
    n_outs = len(out_names)
    all_names = list(in_names) + list(out_names)
    if partition_name is not None:
        all_names.append(partition_name)

    def _body(*args):
        operands = list(args)
        if partition_name is not None:
            operands.append(partition_id_tensor())
        outs = _bass_exec_p.bind(
            *operands,
            out_avals=tuple(out_avals),
            in_names=tuple(all_names),
            out_names=tuple(out_names),
            lowering_input_output_aliases=(),
            sim_require_finite=True,
            sim_require_nnan=True,
            nc=nc,
        )
        return tuple(outs)

    devices = jax.devices()[:NCORES]
    assert len(devices) == NCORES
    mesh = Mesh(np.asarray(devices), ("core",))
    in_specs = (PartitionSpec("core"),) * (n_params + n_outs)
    out_specs = (PartitionSpec("core"),) * n_outs
    donate_argnums = tuple(range(n_params, n_params + n_outs)) if donate else ()
    sharded = jax.jit(
        shard_map(
            _body, mesh=mesh, in_specs=in_specs, out_specs=out_specs, check_rep=False
        ),
        donate_argnums=donate_argnums,
        keep_unused=True,
    )

    return {
        "nc": nc,
        "sharded": sharded,
        "in_names": in_names,
        "out_names": out_names,
        "out_avals": out_avals,
        "n_params": n_params,
        "n_outs": n_outs,
        "partition_name": partition_name,
        "mesh": mesh,
    }


def _get_runner():
    if "runner" not in _CACHE:
        _CACHE["runner"] = _make_sharded(_build(**_SHIP))
    return _CACHE["runner"]


def _prep_arrays(z: np.ndarray, S: np.ndarray, dV: np.ndarray) -> dict:
    """Host-side shard prep: dtype casts + tile-blocked transposed layouts.

    zt [8*NT_S, P, TWS]: core k, tile t, partition p, col j ->
        z[k*L + B*(t*TWS + j) + p - 2]
    st [8*NT_S, B, TWS]: same mapping without the -2 halo offset
    dvt [8*NT_D, P, TWD]: dense row-major per core (H is elementwise)
    wts [8P, 5*128]: stationary matrices, replicated per core
    """
    from numpy.lib.stride_tricks import as_strided

    dt_s = _np_dt("f8e3" if _SHIP.get("s8") else "f16")
    dt_dv = _np_dt("f8e3" if _SHIP.get("dv8") else "f16")
    dt_z = _np_dt("bf16" if _SHIP.get("bf16") else "f16")

    z16 = z.astype(dt_z)
    zp = np.zeros(7 * L + B * N + 256, dt_z)
    zp[2 : 2 + M] = z16  # zp[j] = z[j-2], zeros outside
    zblocks = []
    for k in range(NCORES):
        v = as_strided(zp[k * L :], shape=(N, P), strides=(B * 2, 2))
        # [N, P] -> [NT_S, TWS, P] -> [NT_S, P, TWS]
        zblocks.append(
            np.ascontiguousarray(v.reshape(NT_S, TWS, P).transpose(0, 2, 1))
        )
    zt = np.concatenate(zblocks, axis=0)

    spad = np.zeros(7 * L + B * N + 256, np.float32)
    spad[:M] = S
    s8 = spad.astype(dt_s)
    sblocks = []
    for k in range(NCORES):
        v = s8[k * L : k * L + B * N].reshape(N, B)
        sblocks.append(
            np.ascontiguousarray(v.reshape(NT_S, TWS, B).transpose(0, 2, 1))
        )
    st = np.concatenate(sblocks, axis=0)

    dvt = np.ascontiguousarray(
        dV.astype(dt_dv).reshape(NCORES, P, NT_D, TWD).transpose(0, 2, 1, 3)
    ).reshape(NCORES * NT_D, P, TWD)
    wts = np.tile(_weights_np(bool(_SHIP.get("s8"))).astype(dt_z), (NCORES, 1))
    return {"zt": zt, "st": st, "dvt": dvt, "wts": wts}


def _limiter_scalar(a: np.float32, b: np.float32) -> np.float32:
    x1 = _f32(_f32(abs(_f32(a + b))) * _f32(0.5))
    x2 = _f32(_f32(2.0) * min(_f32(abs(a)), _f32(abs(b))))
    return min(x1, x2)


def _h_exact(v: np.ndarray, dv: np.ndarray) -> np.ndarray:
    """Exact fp32 replica of the reference h_function (for rare V<-54 fixups)."""
    v = v.astype(np.float32)
    dv = dv.astype(np.float32)
    delta_v = np.maximum(_f32(VT) - v, _f32(-1.0))
    T = (delta_v / _f32(SIGMA) / _f32(SQRT2)).astype(np.float32)
    T64 = T.astype(np.float64)
    A = np.exp(
        0.0061 - 1.12 * T64 - 0.257 * T64**2 - 0.072 * T64**3 - 0.0117 * T64**4
    ).astype(np.float32)
    dT_dt = np.minimum(_f32(_C2) * dv, _f32(0.0)).astype(np.float32)
    erf = np.vectorize(math.erf)(T64)
    F_T = (SQRT_2_PI * np.exp(-(T64**2)) / (1.00000001 + erf)).astype(np.float32)
    B_ = (_f32(-SQRT2) * dT_dt * F_T * _f32(TAU_M)).astype(np.float32)
    return np.maximum((A + B_) / _f32(TAU_M), _f32(0.0)).astype(np.float32)


def kernel(z, Sourse, V, dVdt) -> np.ndarray:
    z = np.ascontiguousarray(np.asarray(z, dtype=np.float32))
    S = np.ascontiguousarray(np.asarray(Sourse, dtype=np.float32))
    V = np.asarray(V, dtype=np.float32)
    dV = np.ascontiguousarray(np.asarray(dVdt, dtype=np.float32))
    assert z.shape == (M,)

    r = _get_runner()
    arrs = _prep_arrays(z, S, dV)
    ins = [arrs[name] for name in r["in_names"]]
    zeros = [
        np.zeros((NCORES * av.shape[0], *av.shape[1:]), av.dtype)
        for av in r["out_avals"]
    ]
    out_arrs = r["sharded"](*ins, *zeros)
    by_name = dict(zip(r["out_names"], out_arrs))

    out = np.empty((2, M), np.float32)
    dz_t = np.asarray(by_name["dz"])  # [8*NT_S, B, TWS] fp16
    for k in range(NCORES):
        blk = dz_t[k * NT_S : (k + 1) * NT_S]  # [NT_S, B, TWS]
        cols = blk.transpose(0, 2, 1).reshape(N, B)  # [global col, p]
        out[0, k * L : (k + 1) * L] = cols.reshape(-1)[:L].astype(np.float32)
    ho_t = np.asarray(by_name["ho"])  # [8*NT_D, P, TWD]
    out[1] = (
        ho_t.reshape(NCORES, NT_D, P, TWD)
        .transpose(0, 2, 1, 3)
        .reshape(M)
        .astype(np.float32)
    )

    # ---- exact host fixups for the 3 boundary dz elements ----
    z0, z1, z2_ = _f32(z[0]), _f32(z[1]), _f32(z[2])
    s0, s1 = _f32(S[0]), _f32(S[1])
    out[0, 0] = _f32(_f32(_f32(-2.0) * z0) - s0)
    d0 = _f32(z1 - z0)
    d1 = _f32(z2_ - z1)
    w1 = _limiter_scalar(d1, d0)
    t = _f32(_COEF32 * _f32(w1 - _f32(0.0)))
    out[0, 1] = _f32(_f32(_f32(-2.0) * _f32(d0 + t)) - s1)
    zm1, zm2, zm3 = _f32(z[M - 1]), _f32(z[M - 2]), _f32(z[M - 3])
    wl = _limiter_scalar(_f32(zm1 - zm2), _f32(zm2 - zm3))
    out[0, M - 1] = _f32(
        _f32(_f32(2.0) * _f32(zm2 + _f32(_COEF32 * wl))) - _f32(S[M - 1])
    )

    # ---- H fixup for any V < -54 (delta_V != -1); never triggers for randn ----
    bad = np.flatnonzero(V < _f32(-54.0))
    if bad.size:
        out[1, bad] = _h_exact(V[bad], dV[bad])

    return out
